# revision 59
# baseline (speedup 1.0000x reference)
"""Sparse (conv-compressed) multi-head attention on 8 Trainium2 NeuronCores.

Entry point: kernel(**inputs) -> np.ndarray [4, 2048, 1024] float32.

Sharding: core c = 2*b + g  (b = batch 0..3, g = head-half 0..1).
Each core: batch b, heads [8g, 8g+8), all 2048 queries.
Final projection produces a partial (dv-half contraction); host sums pairs + bias.

v3 design notes:
- The strided conv that builds the compressed keys kc is FUSED into the
  Wk/Wv projections on the host:  kp = (Wk_hh @ Wc3) @ k3,  vp likewise,
  where k3(t) = concat(k[3t], k[3t+1], k[3t+2]).  No conv intermediate,
  no cross-core collective.  Column j=0 (kc[0] = k[0]) comes from the host.
- The host ships kT pre-decimated as K3T[r, i, j] = k[3(j-1)+r, i] so every
  matmul operand is contiguous (strided SBUF reads cost ~1.5x on the PE).
- Score matmuls + exp are decoupled from the attention-value matmuls: scores
  stream into the qp projection passes so the scalar engine's exp pipeline
  (the second-longest serial resource) starts ~60us earlier and never paces
  the PE.  o/normalize/output-projection work is interleaved per q-tile at
  the tail.
- Causal mask applied as a 0/1 bf16 multiply on eT after exp (gpsimd).
"""
import sys
sys.path.insert(0, '/opt/trn_rl_repo')
import numpy as np
import concourse.bass as bass
import concourse.bacc as bacc
import concourse.mybir as mybir
from concourse import tile
from contextlib import ExitStack

f32 = mybir.dt.float32
f32r = mybir.dt.float32r
bf16 = mybir.dt.bfloat16
DT = bf16
Exp = mybir.ActivationFunctionType.Exp

B, T, D, H = 4, 2048, 1024, 16
DH = 64
TC = 683          # compressed keys: 1 + 682
DHALF = D // 2    # per-core head-half width
H8 = H // 2       # heads per core
SCALE = DH ** -0.5  # 0.125

# kp n-splits over keys [1, 683)
KPN = [(1, 341), (342, 341)]

# attention q-tiles (4 x 512) and j-chunks (6 x 128, last = 43 rows)
NJ = 6
JROWS = [128, 128, 128, 128, 128, TC - 5 * 128]  # last = 43
JCS = {m: [jc for jc in range(NJ) if 384 * jc < 512 * (m + 1)] for m in range(4)}
C0 = {(m, jc): max(0, 384 * jc - 512 * m) for m in range(4) for jc in JCS[m]}
# ragged tiles (m, jc) -> first fully-visible column c1 (cols [c0, c1) get mask)
RAGGED = {}
for m in range(4):
    for jc in JCS[m]:
        if not (384 * jc + 381 <= 512 * m):
            RAGGED[(m, jc)] = min(512, 384 * jc + 381 - 512 * m)
RAGGED_LIST = sorted(RAGGED.keys())  # 8 tiles
assert len(RAGGED_LIST) == 8


def chunk_even(lst, n):
    """Split lst into n chunks with sizes as even as possible."""
    k, r = divmod(len(lst), n)
    out, i = [], 0
    for j in range(n):
        sz = k + (1 if j < r else 0)
        out.append(lst[i:i + sz])
        i += sz
    return out


def build_nc():
    nc = bacc.Bacc(None, target_bir_lowering=False, debug=False)

    qT = nc.dram_tensor("qT", [D, T], DT, kind="ExternalInput")
    K3T = nc.dram_tensor("K3T", [3, D, TC], DT, kind="ExternalInput")
    WEK = nc.dram_tensor("WEK", [3 * D, DHALF], DT, kind="ExternalInput")
    WEV = nc.dram_tensor("WEV", [3 * D, DHALF], DT, kind="ExternalInput")
    WQT = nc.dram_tensor("WQT", [D, DHALF], DT, kind="ExternalInput")
    WOT = nc.dram_tensor("WOT", [DHALF, D], DT, kind="ExternalInput")
    MASKB = nc.dram_tensor("MASKB", [8, 128, 512], DT, kind="ExternalInput")
    SEL4 = nc.dram_tensor("SEL4", [4, 128, 128], f32r, kind="ExternalInput")
    KP0 = nc.dram_tensor("KP0", [DHALF, 1], DT, kind="ExternalInput")
    VP0R = nc.dram_tensor("VP0R", [1, 520], DT, kind="ExternalInput")
    OUT = nc.dram_tensor("out_p", [T, D], f32, kind="ExternalOutput")

    with tile.TileContext(nc) as tc, ExitStack() as st:
        st.enter_context(nc.allow_low_precision("bf16 matmuls, f32r recip bcast"))
        pool = lambda **kw: st.enter_context(tc.tile_pool(**kw))
        p_k3 = pool(name="k3", bufs=24)       # K3T chunks [128, 683]
        p_wk = pool(name="wk", bufs=6)        # WEK chunks [128, 512]
        p_wv = pool(name="wv", bufs=6)        # WEV chunks [128, 512]
        p_wq = pool(name="wq", bufs=8)        # WQT chunks [128, 512]
        p_qt = pool(name="qt", bufs=8)        # qT chunks [128, 512]
        p_kpt = pool(name="kpt", bufs=8)      # kp^T per head, K zero-padded
        p_vpa = pool(name="vpa", bufs=6)      # vp_aug [128, 520]
        p_qpt = pool(name="qpt", bufs=4)      # qp^T [128, 2048]
        p_mask = pool(name="mask", bufs=8)    # 0/1 bf16 masks [128, 512]
        p_et = pool(name="et", bufs=42)       # exp(scores) [128, 512] bf16
        p_ont = pool(name="ont", bufs=16)     # unnormalized head outs [128, 512]
        p_wo = pool(name="wo", bufs=8)        # WoT resident [128, 512]
        p_out = pool(name="outsb", bufs=3)    # out staging [128, 1024] f32
        p_rm = pool(name="rm", bufs=4)        # per-m recip denominators [8, 512]
        p_ss = pool(name="ss", bufs=8)        # denom staging rows [1, 512]
        p_small = pool(name="small", bufs=6)

        # kpZ[h]: rows 0-63 = head h's kp dims, rows 64-127 = ZERO.  Scores
        # then contract K=128 (full-rate: K=64 matmuls stream at half speed);
        # the zero rows null out the other head's qp rows in the rhs.
        # head h lives in qpt partition rows [64*(h%2), +64); kpZ mirrors that
        # row placement and zeroes the other head's rows
        kpZ = [p_kpt.tile([128, TC], DT, name="kpt", tag="kpt") for _ in range(H8)]
        for h in range(H8):
            z0 = 64 * ((h + 1) % 2)
            nc.vector.memset(kpZ[h][z0:z0 + 64, :], 0.0)
        vpa = [p_vpa.tile([128, 520], DT, name="vpa", tag="vpa") for _ in range(NJ)]
        qpt = [p_qpt.tile([128, T], DT, name="qpt", tag="qpt") for _ in range(4)]
        o_nt = {(kk, m): p_ont.tile([128, 512], DT, name="ont", tag="ont")
                for kk in range(4) for m in range(4)}
        ones_vpa = p_small.tile([128, 8], f32, name="ones_vpa", tag="ones_vpa",
                                bufs=1)
        nc.vector.memset(ones_vpa[:], 1.0)

        # ---- kp^T fused: kpt[m][:, j] = (Wk_hh @ Wc3) @ k3(j-1), j in [1, 683)
        k3 = {}
        k3q = [nc.sync, nc.scalar, nc.gpsimd]  # parallel issue for fast start
        with tc.tile_pool(name="ps_kp", bufs=8, space="PSUM") as ps_pool:
            ps_kp = {}
            for ic in range(8):
                for r in range(3):
                    kt = p_k3.tile([128, TC], DT, name="k3", tag="k3")
                    eng = k3q[r] if ic == 0 else nc.sync
                    if ic == 0 and r == 0:
                        # split the critical first chunk so the first matmuls
                        # start as soon as the leading halves land
                        eng.dma_start(kt[:, 0:342], K3T[r, 0:128, 0:342])
                        eng.dma_start(kt[:, 342:], K3T[r, 0:128, 342:])
                    else:
                        eng.dma_start(kt[:], K3T[r, 128 * ic:128 * (ic + 1), :])
                    k3[(ic, r)] = kt
                    wt = p_wk.tile([128, DHALF], DT, name="wk", tag="wk")
                    if ic == 0 and r == 0:
                        for mw in range(4):
                            nc.gpsimd.dma_start(
                                wt[:, 128 * mw:128 * (mw + 1)],
                                WEK[1024 * r:1024 * r + 128,
                                    128 * mw:128 * (mw + 1)])
                    else:
                        nc.gpsimd.dma_start(
                            wt[:],
                            WEK[1024 * r + 128 * ic:1024 * r + 128 * (ic + 1), :])
                    first = (ic == 0 and r == 0)
                    last = (ic == 7 and r == 2)
                    for m in range(4):
                        for ni, (t0, tw) in enumerate(KPN):
                            if first:
                                ps_kp[(m, ni)] = ps_pool.tile(
                                    [128, 341], f32, name="ps_kp", tag="ps_kp")
                            nc.tensor.matmul(
                                ps_kp[(m, ni)][:, :tw],
                                wt[:, 128 * m:128 * (m + 1)],
                                kt[:, t0:t0 + tw],
                                start=first, stop=last)
            for (m, ni), ps in ps_kp.items():
                t0, tw = KPN[ni]
                # feature rows [0:64] = head 2m, [64:128] = head 2m+1; rows
                # keep their partition placement (no shift needed)
                nc.scalar.copy(kpZ[2 * m][0:64, t0:t0 + tw], ps[0:64, :tw])
                nc.vector.tensor_copy(
                    kpZ[2 * m + 1][64:128, t0:t0 + tw], ps[64:128, :tw])

        # ---- small loads on the scalar queue, issued after the kp phase so
        # they don't compete with K3T/WEK for HBM bandwidth at startup
        mk = []
        for ti in range(8):
            mt = p_mask.tile([128, 512], DT, name="mask", tag="mask")
            nc.scalar.dma_start(mt[:], MASKB[ti])
            mk.append(mt)
        sel = []
        for t4 in range(4):
            s_t = p_small.tile([128, 128], f32r, name="sel", tag="sel", bufs=4)
            nc.scalar.dma_start(s_t[:], SEL4[t4])
            sel.append(s_t)
        wot = {}
        for kk in range(4):
            for nn2 in range(2):
                wt = p_wo.tile([128, 512], DT, name="wo", tag="wo")
                nc.scalar.dma_start(
                    wt[:], WOT[128 * kk:128 * (kk + 1), 512 * nn2:512 * (nn2 + 1)])
                wot[(kk, nn2)] = wt
        vp0 = p_small.tile([1, 520], DT, name="vp0", tag="vp0", bufs=1)
        nc.scalar.dma_start(vp0[:], VP0R[:])
        with nc.allow_non_contiguous_dma(reason="kp col-0 writes, 1KB total"):
            for h in range(H8):
                off = 64 * (h % 2)
                nc.scalar.dma_start(
                    kpZ[h][off:off + 64, 0:1], KP0[64 * h:64 * (h + 1), :])

        # ---- vp_aug fused: vpa[jb][j, 65h+c] = vp[128jb+j, 64h+c], col 65h+64=1
        with tc.tile_pool(name="ps_vp", bufs=6, space="PSUM") as ps_pool:
            ps_vp = {}
            for ic in range(8):
                for r in range(3):
                    wt = p_wv.tile([128, DHALF], DT, name="wv", tag="wv")
                    nc.gpsimd.dma_start(
                        wt[:], WEV[1024 * r + 128 * ic:1024 * r + 128 * (ic + 1), :])
                    first = (ic == 0 and r == 0)
                    last = (ic == 7 and r == 2)
                    for jb in range(NJ):
                        jr = JROWS[jb]
                        if first:
                            ps_vp[jb] = ps_pool.tile(
                                [128, 512], f32, name="ps_vp", tag="ps_vp")
                        nc.tensor.matmul(
                            ps_vp[jb][:jr, :],
                            k3[(ic, r)][:, 128 * jb:128 * jb + jr],
                            wt[:],
                            start=first, stop=last)
            for jb in range(NJ):
                jr = JROWS[jb]
                dst = vpa[jb][:jr].rearrange("p (h c) -> p h c", c=65)
                src = ps_vp[jb][:jr].rearrange("p (h c) -> p h c", c=64)
                if jb % 2 == 0:
                    nc.scalar.copy(dst[:, :, 0:64], src[:])
                else:
                    nc.vector.tensor_copy(dst[:, :, 0:64], src[:])
                nc.vector.tensor_copy(
                    dst[:, :, 64:65],
                    ones_vpa[:jr, :].rearrange("p (h c) -> p h c", c=1))
            # row 0 (key 0) comes from the host: kc[0] = k[0]
            nc.vector.tensor_copy(vpa[0][0:1, :], vp0[:])

        # ---- attention helpers (scores decoupled from o) ----
        ET = {}

        def emit_score(m, h, jc, ps_sc, s_bufs=2):
            hc = h // 2
            jr, c0 = JROWS[jc], C0[(m, jc)]
            ps = ps_sc.tile([128, 512], f32, name="ps_s", tag="ps_s",
                            bufs=s_bufs)
            nc.tensor.matmul(
                ps[:jr, c0:],
                kpZ[h][:, 128 * jc:128 * jc + jr],
                qpt[hc][:, 512 * m + c0:512 * (m + 1)],
                start=True, stop=True)
            et = p_et.tile([128, 512], DT, name="et", tag="et")
            nc.scalar.activation(et[:jr, c0:], ps[:jr, c0:], Exp, scale=SCALE)
            if (m, jc) in RAGGED:
                c1 = RAGGED[(m, jc)]
                ti = RAGGED_LIST.index((m, jc))
                nc.gpsimd.tensor_mul(
                    et[:jr, c0:c1], et[:jr, c0:c1], mk[ti][:jr, c0:c1])
            ET[(m, h, jc)] = et

        # rm zero-padded to K=128 for the full-rate bc matmul (sel rows >= 8
        # are zero, but NaN garbage in rm rows would still poison 0*NaN)
        rm = [p_rm.tile([128, 512], f32r, name="rm", tag="rm") for _ in range(4)]
        for m in range(4):
            nc.vector.memset(rm[m][:].bitcast(f32), 0.0)
        S_m = [p_rm.tile([8, 512], f32, name="sm", tag="sm") for _ in range(4)]

        def emit_o_mm(m, h, jc, po, first, last):
            jr, c0 = JROWS[jc], C0[(m, jc)]
            nc.tensor.matmul(
                po[:65, c0:],
                vpa[jc][:jr, 65 * h:65 * (h + 1)],
                ET[(m, h, jc)][:jr, c0:],
                start=first, stop=last)

        def emit_o_post(m, h, po):
            hc, off = h // 2, (h % 2) * 64
            ss = p_ss.tile([1, 512], f32, name="ss", tag="ss", bufs=2)
            nc.vector.tensor_copy(ss[:], po[64:65, :])
            nc.sync.dma_start(S_m[m][h:h + 1, :], ss[:])
            nc.vector.tensor_copy(o_nt[(hc, m)][off:off + 64, :], po[0:64, :])
            if h == H8 - 1:
                rt = p_ss.tile([8, 512], f32, name="rt", tag="rt", bufs=1)
                nc.vector.reciprocal_approx_fast(out=rt[:], in_=S_m[m][:])
                nc.vector.tensor_copy(rm[m][0:8, :], rt[:])

        def emit_o(m, h, ps_att, o_bufs=2):
            js = JCS[m]
            po = ps_att.tile([128, 512], f32, name="ps_o", tag="ps_o",
                             bufs=o_bufs)
            for jc in js:
                emit_o_mm(m, h, jc, po, jc == js[0], jc == js[-1])
            emit_o_post(m, h, po)

        def emit_bc(m, ps_att):
            for t4 in range(4):
                bc = ps_att.tile([128, 512], f32, name="ps_bc", tag="ps_bc",
                                 bufs=2)
                nc.tensor.matmul(bc[:], sel[t4][:], rm[m][:], start=True,
                                 stop=True)
                for half in range(2):
                    dst = o_nt[(t4, m)][64 * half:64 * half + 64, :]
                    nc.vector.tensor_mul(
                        dst, dst, bc[64 * half:64 * half + 64, :])

        def emit_fin_mq(m, mq, ps_att):
            ob = p_out.tile([128, D], f32, name="outsb", tag="outsb")
            rows = slice(512 * m + 128 * mq, 512 * m + 128 * (mq + 1))
            for nn2 in range(2):
                pf = ps_att.tile([128, 512], f32, name="ps_f", tag="ps_f",
                                 bufs=2)
                for kk in range(4):
                    nc.tensor.matmul(
                        pf[:],
                        o_nt[(kk, m)][:, 128 * mq:128 * (mq + 1)],
                        wot[(kk, nn2)][:],
                        start=(kk == 0), stop=(kk == 3))
                # scalar still runs the exp stream while m=0/1 finalize
                if m < 2 or nn2 == 1:
                    nc.vector.tensor_copy(ob[:, 512 * nn2:512 * (nn2 + 1)], pf[:])
                else:
                    nc.scalar.copy(ob[:, 512 * nn2:512 * (nn2 + 1)], pf[:])
            if m == 3 and mq == 3:
                # final chunk: two parallel half-DMAs shorten the drain tail
                nc.gpsimd.dma_start(OUT[rows, 0:512], ob[:, 0:512])
                nc.sync.dma_start(OUT[rows, 512:1024], ob[:, 512:1024])
            else:
                dq = (nc.gpsimd, nc.sync)[mq % 2]
                dq.dma_start(OUT[rows, :], ob[:])

        # ---- qp passes; scores(m=pass-1) and o(pass-2) interleaved ----
        # PSUM per pass: ps_qp 4 + ps_s 2 + ps_o 2 = 8 banks.
        with tc.tile_pool(name="ps_qp", bufs=8, space="PSUM") as ps_pool:
            for npass in range(4):
                sc_chunks = [[] for _ in range(8)]
                if npass >= 1:
                    msc = npass - 1
                    tiles = [(h, jc) for h in range(H8) for jc in JCS[msc]]
                    sc_chunks = chunk_even(tiles, 8)
                ps_qp = {}
                for kk in range(8):
                    wq_t = p_wq.tile([128, DHALF], DT, name="wq", tag="wq")
                    nc.sync.dma_start(wq_t[:], WQT[128 * kk:128 * (kk + 1), :])
                    qt_t = p_qt.tile([128, 512], DT, name="qt", tag="qt")
                    nc.sync.dma_start(
                        qt_t[:],
                        qT[128 * kk:128 * (kk + 1), 512 * npass:512 * (npass + 1)])
                    # zip qp matmuls with score tiles so the exp stream always
                    # has non-dependent PE work between score matmuls
                    sc_it = list(sc_chunks[kk])
                    for m in range(4):
                        if kk == 0:
                            ps_qp[m] = ps_pool.tile(
                                [128, 512], f32, name="ps_qp", tag="ps_qp",
                                bufs=4)
                        nc.tensor.matmul(
                            ps_qp[m][:],
                            wq_t[:, 128 * m:128 * (m + 1)],
                            qt_t[:],
                            start=(kk == 0), stop=(kk == 7))
                        if sc_it:
                            h, jc = sc_it.pop(0)
                            emit_score(npass - 1, h, jc, ps_pool, s_bufs=3)
                    for (h, jc) in sc_it:
                        emit_score(npass - 1, h, jc, ps_pool, s_bufs=3)
                    if npass >= 2:
                        emit_o(npass - 2, kk, ps_pool, o_bufs=1)
                for m, ps in ps_qp.items():
                    if m % 2 == 0:
                        nc.scalar.copy(
                            qpt[m][:, 512 * npass:512 * (npass + 1)], ps[:])
                    else:
                        nc.vector.tensor_copy(
                            qpt[m][:, 512 * npass:512 * (npass + 1)], ps[:])

        # ---- tail: sc(3) / o(2) / o(3) / finalize pipelined per head ----
        # PSUM: ps_s 2 + ps_o 2 + ps_bc 2 + ps_f 2 = 8 banks.
        with tc.tile_pool(name="ps_att", bufs=2, space="PSUM") as ps_att:
            fin_chunks = []  # deferred finalize chunks for m = 0, 1
            fin_chunks.append(lambda: emit_bc(0, ps_att))
            for mq in range(4):
                fin_chunks.append(lambda m=0, q=mq: emit_fin_mq(m, q, ps_att))
            fin_chunks.append(lambda: emit_bc(1, ps_att))
            for mq in range(4):
                fin_chunks.append(lambda m=1, q=mq: emit_fin_mq(m, q, ps_att))
            ci = 0
            for h in range(H8):
                # zip sc3 / o2 / o3 matmuls so exp latency hides behind
                # independent o-matmuls instead of gating ps_s recycling
                js3, js2 = JCS[3], JCS[2]
                po2 = ps_att.tile([128, 512], f32, name="ps_o", tag="ps_o",
                                  bufs=2)
                emit_score(3, h, js3[0], ps_att)
                emit_score(3, h, js3[1], ps_att)
                for i, jc in enumerate(js2):
                    emit_o_mm(2, h, jc, po2, i == 0, i == len(js2) - 1)
                emit_score(3, h, js3[2], ps_att)
                emit_score(3, h, js3[3], ps_att)
                if h >= 1:
                    po3 = ps_att.tile([128, 512], f32, name="ps_o",
                                      tag="ps_o", bufs=2)
                    for i, jc in enumerate(js3):
                        emit_o_mm(3, h - 1, jc, po3, i == 0, i == 5)
                emit_score(3, h, js3[4], ps_att)
                emit_score(3, h, js3[5], ps_att)
                emit_o_post(2, h, po2)
                if h >= 1:
                    emit_o_post(3, h - 1, po3)
                # 1-2 finalize chunks per slot; bc(1) needs o(1) (done in qp
                # pass 3), all fit behind the sc3/o2/o3 stream
                for _ in range(2 if h >= 4 else 1):
                    if ci < len(fin_chunks):
                        fin_chunks[ci]()
                        ci += 1
            while ci < len(fin_chunks):
                fin_chunks[ci]()
                ci += 1
            emit_bc(2, ps_att)
            emit_o(3, 7, ps_att)
            for mq in range(4):
                emit_fin_mq(2, mq, ps_att)
            # fin(3): all 4 bc tiles live at once (borrow the now-idle ps_s /
            # ps_o rings), normalize per 128-col slice, interleave with the
            # output-projection matmuls to shorten the tail
            bc3 = []
            for t4, tag in enumerate(("ps_bc", "ps_bc", "ps_s", "ps_o")):
                bc = ps_att.tile([128, 512], f32, name="ps_bc3", tag=tag,
                                 bufs=2)
                nc.tensor.matmul(bc[:], sel[t4][:], rm[3][:], start=True,
                                 stop=True)
                bc3.append(bc)
            for mq in range(4):
                for t4 in range(4):
                    for half in range(2):
                        dst = o_nt[(t4, 3)][64 * half:64 * half + 64,
                                            128 * mq:128 * (mq + 1)]
                        nc.vector.tensor_mul(
                            dst, dst,
                            bc3[t4][64 * half:64 * half + 64,
                                    128 * mq:128 * (mq + 1)])
                emit_fin_mq(3, mq, ps_att)

    return nc


def make_maskb():
    import ml_dtypes
    mask = np.zeros((8, 128, 512), dtype=np.float32)
    for t, (m, jc) in enumerate(RAGGED_LIST):
        qq = 512 * m + np.arange(512)[None, :]
        jj = 128 * jc + np.arange(128)[:, None]
        mask[t] = (3 * jj <= qq).astype(np.float32)  # 1.0 where visible
    return mask.astype(ml_dtypes.bfloat16)


def make_sel4():
    sel = np.zeros((4, 128, 128), dtype=np.float32)
    for t in range(4):
        for r in range(128):
            sel[t, 2 * t + r // 64, r] = 1.0
    return sel


def make_k3t(kb):
    """kb: [T, D] f32 for one batch -> K3T [3, D, TC] bf16.
    K3T[r, i, j] = k[3(j-1)+r, i] for j >= 1; column 0 is zero."""
    import ml_dtypes
    out = np.zeros((3, D, TC), dtype=np.float32)
    for r in range(3):
        out[r, :, 1:] = kb[r:r + 3 * (TC - 1):3, :].T
    return out.astype(ml_dtypes.bfloat16)


def prep_inputs(q, k, Wq, Wk, Wv, Wo, conv_w):
    """Returns list of 8 in_maps (core c = 2b + g)."""
    import ml_dtypes
    bf = ml_dtypes.bfloat16
    # Wc3T[r*1024 + i, o] = conv_w[o, i, r]  (so kc[j] = Wc3T.T @ k3(j-1))
    Wc3T = np.ascontiguousarray(
        conv_w.transpose(2, 1, 0).reshape(3 * D, D)).astype(np.float32)
    maskb = make_maskb()
    sel4 = make_sel4()
    halves = []
    for g in range(2):
        sl = slice(DHALF * g, DHALF * (g + 1))
        WEKg = np.ascontiguousarray((Wc3T @ Wk[sl].T)).astype(bf)
        WEVg = np.ascontiguousarray((Wc3T @ Wv[sl].T)).astype(bf)
        halves.append((sl, WEKg, WEVg))
    k3ts = [make_k3t(k[b]) for b in range(B)]
    in_maps = []
    for c in range(8):
        b, g = c // 2, c % 2
        sl, WEKg, WEVg = halves[g]
        kp0 = (Wk[sl] @ k[b, 0]).astype(np.float32).reshape(DHALF, 1)
        vp0 = (Wv[sl] @ k[b, 0]).astype(np.float32)
        vp0r = np.zeros((8, 65), np.float32)
        vp0r[:, :64] = vp0.reshape(8, 64)
        vp0r[:, 64] = 1.0
        in_maps.append({
            "qT": np.ascontiguousarray(q[b].T).astype(bf),
            "K3T": k3ts[b],
            "WEK": WEKg,
            "WEV": WEVg,
            "WQT": np.ascontiguousarray(Wq[sl, :].T).astype(bf),
            "WOT": np.ascontiguousarray(Wo[:, sl].T).astype(bf),
            "MASKB": maskb,
            "SEL4": sel4,
            "KP0": kp0.astype(bf),
            "VP0R": vp0r.reshape(1, 520).astype(bf),
        })
    return in_maps


def postprocess(results, bo):
    out = np.zeros((B, T, D), dtype=np.float32)
    for b in range(B):
        out[b] = (np.asarray(results[2 * b]["out_p"], dtype=np.float32)
                  + np.asarray(results[2 * b + 1]["out_p"], dtype=np.float32)
                  + bo[None, :])
    return out


_CACHED_NC = None


def kernel(q, k, v, Wq, Wk, Wv, Wo, bo, conv_w):
    """Full-input entry point. v is unused by the reference computation
    (V is replaced by the conv-compressed K)."""
    global _CACHED_NC
    from concourse.bass_utils import run_bass_kernel_spmd

    q = np.asarray(q, dtype=np.float32)
    k = np.asarray(k, dtype=np.float32)
    Wq = np.asarray(Wq, dtype=np.float32)
    Wk = np.asarray(Wk, dtype=np.float32)
    Wv = np.asarray(Wv, dtype=np.float32)
    Wo = np.asarray(Wo, dtype=np.float32)
    bo = np.asarray(bo, dtype=np.float32)
    conv_w = np.asarray(conv_w, dtype=np.float32)

    in_maps = prep_inputs(q, k, Wq, Wk, Wv, Wo, conv_w)
    if _CACHED_NC is None:
        nc = build_nc()
        nc.finalize()
        _CACHED_NC = nc
    res = run_bass_kernel_spmd(_CACHED_NC, in_maps, list(range(8)))
    return postprocess(res.results, bo)


# revision 61
# speedup vs baseline: 1.0166x; 1.0166x over previous
"""Sparse (conv-compressed) multi-head attention on 8 Trainium2 NeuronCores.

Entry point: kernel(**inputs) -> np.ndarray [4, 2048, 1024] float32.

Sharding: core c = 2*b + g  (b = batch 0..3, g = head-half 0..1).
Each core: batch b, heads [8g, 8g+8), all 2048 queries.
Final projection produces a partial (dv-half contraction); host sums pairs + bias.

v3 design notes:
- The strided conv that builds the compressed keys kc is FUSED into the
  Wk/Wv projections on the host:  kp = (Wk_hh @ Wc3) @ k3,  vp likewise,
  where k3(t) = concat(k[3t], k[3t+1], k[3t+2]).  No conv intermediate,
  no cross-core collective.  Column j=0 (kc[0] = k[0]) comes from the host.
- The host ships kT pre-decimated as K3T[r, i, j] = k[3(j-1)+r, i] so every
  matmul operand is contiguous (strided SBUF reads cost ~1.5x on the PE).
- Score matmuls + exp are decoupled from the attention-value matmuls: scores
  stream into the qp projection passes so the scalar engine's exp pipeline
  (the second-longest serial resource) starts ~60us earlier and never paces
  the PE.  o/normalize/output-projection work is interleaved per q-tile at
  the tail.
- Causal mask applied as a 0/1 bf16 multiply on eT after exp (gpsimd).
"""
import sys
sys.path.insert(0, '/opt/trn_rl_repo')
import numpy as np
import concourse.bass as bass
import concourse.bacc as bacc
import concourse.mybir as mybir
from concourse import tile
from contextlib import ExitStack

f32 = mybir.dt.float32
f32r = mybir.dt.float32r
bf16 = mybir.dt.bfloat16
DT = bf16
Exp = mybir.ActivationFunctionType.Exp

B, T, D, H = 4, 2048, 1024, 16
DH = 64
TC = 683          # compressed keys: 1 + 682
DHALF = D // 2    # per-core head-half width
H8 = H // 2       # heads per core
SCALE = DH ** -0.5  # 0.125

# kp n-splits over keys [1, 683)
KPN = [(1, 341), (342, 341)]

# attention q-tiles (4 x 512) and j-chunks (6 x 128, last = 43 rows)
NJ = 6
JROWS = [128, 128, 128, 128, 128, TC - 5 * 128]  # last = 43
JCS = {m: [jc for jc in range(NJ) if 384 * jc < 512 * (m + 1)] for m in range(4)}
C0 = {(m, jc): max(0, 384 * jc - 512 * m) for m in range(4) for jc in JCS[m]}
# ragged tiles (m, jc) -> first fully-visible column c1 (cols [c0, c1) get mask)
RAGGED = {}
for m in range(4):
    for jc in JCS[m]:
        if not (384 * jc + 381 <= 512 * m):
            RAGGED[(m, jc)] = min(512, 384 * jc + 381 - 512 * m)
RAGGED_LIST = sorted(RAGGED.keys())  # 8 tiles
assert len(RAGGED_LIST) == 8


def chunk_even(lst, n):
    """Split lst into n chunks with sizes as even as possible."""
    k, r = divmod(len(lst), n)
    out, i = [], 0
    for j in range(n):
        sz = k + (1 if j < r else 0)
        out.append(lst[i:i + sz])
        i += sz
    return out


def build_nc():
    nc = bacc.Bacc(None, target_bir_lowering=False, debug=False)

    qT = nc.dram_tensor("qT", [D, T], DT, kind="ExternalInput")
    K3T = nc.dram_tensor("K3T", [3, D, TC], DT, kind="ExternalInput")
    WEK = nc.dram_tensor("WEK", [3 * D, DHALF], DT, kind="ExternalInput")
    WEV = nc.dram_tensor("WEV", [3 * D, DHALF], DT, kind="ExternalInput")
    WQT = nc.dram_tensor("WQT", [D, DHALF], DT, kind="ExternalInput")
    WOT = nc.dram_tensor("WOT", [DHALF, D], DT, kind="ExternalInput")
    MASKB = nc.dram_tensor("MASKB", [8, 128, 512], DT, kind="ExternalInput")
    SEL4 = nc.dram_tensor("SEL4", [4, 128, 128], f32r, kind="ExternalInput")
    KP0 = nc.dram_tensor("KP0", [DHALF, 1], DT, kind="ExternalInput")
    VP0R = nc.dram_tensor("VP0R", [1, 520], DT, kind="ExternalInput")
    OUT = nc.dram_tensor("out_p", [T, D], f32, kind="ExternalOutput")

    with tile.TileContext(nc) as tc, ExitStack() as st:
        st.enter_context(nc.allow_low_precision("bf16 matmuls, f32r recip bcast"))
        pool = lambda **kw: st.enter_context(tc.tile_pool(**kw))
        p_k3 = pool(name="k3", bufs=24)       # K3T chunks [128, 683]
        p_wk = pool(name="wk", bufs=6)        # WEK chunks [128, 512]
        p_wv = pool(name="wv", bufs=6)        # WEV chunks [128, 512]
        p_wq = pool(name="wq", bufs=8)        # WQT chunks [128, 512]
        p_qt = pool(name="qt", bufs=8)        # qT chunks [128, 512]
        p_kpt = pool(name="kpt", bufs=8)      # kp^T per head, K zero-padded
        p_vpa = pool(name="vpa", bufs=6)      # vp_aug [128, 520]
        p_qpt = pool(name="qpt", bufs=4)      # qp^T [128, 2048]
        p_mask = pool(name="mask", bufs=8)    # 0/1 bf16 masks [128, 512]
        p_et = pool(name="et", bufs=42)       # exp(scores) [128, 512] bf16
        p_ont = pool(name="ont", bufs=16)     # unnormalized head outs [128, 512]
        p_wo = pool(name="wo", bufs=8)        # WoT resident [128, 512]
        p_out = pool(name="outsb", bufs=3)    # out staging [128, 1024] f32
        p_rm = pool(name="rm", bufs=4)        # per-m recip denominators [8, 512]
        p_ss = pool(name="ss", bufs=8)        # denom staging rows [1, 512]
        p_small = pool(name="small", bufs=6)

        # kpZ[h]: rows 0-63 = head h's kp dims, rows 64-127 = ZERO.  Scores
        # then contract K=128 (full-rate: K=64 matmuls stream at half speed);
        # the zero rows null out the other head's qp rows in the rhs.
        # head h lives in qpt partition rows [64*(h%2), +64); kpZ mirrors that
        # row placement and zeroes the other head's rows
        kpZ = [p_kpt.tile([128, TC], DT, name="kpt", tag="kpt") for _ in range(H8)]
        for h in range(H8):
            z0 = 64 * ((h + 1) % 2)
            nc.vector.memset(kpZ[h][z0:z0 + 64, :], 0.0)
        vpa = [p_vpa.tile([128, 520], DT, name="vpa", tag="vpa") for _ in range(NJ)]
        qpt = [p_qpt.tile([128, T], DT, name="qpt", tag="qpt") for _ in range(4)]
        o_nt = {(kk, m): p_ont.tile([128, 512], DT, name="ont", tag="ont")
                for kk in range(4) for m in range(4)}
        ones_vpa = p_small.tile([128, 8], f32, name="ones_vpa", tag="ones_vpa",
                                bufs=1)
        nc.vector.memset(ones_vpa[:], 1.0)

        # ---- kp^T fused: kpt[m][:, j] = (Wk_hh @ Wc3) @ k3(j-1), j in [1, 683)
        k3 = {}
        k3q = [nc.sync, nc.scalar, nc.gpsimd]  # parallel issue for fast start
        with tc.tile_pool(name="ps_kp", bufs=8, space="PSUM") as ps_pool:
            ps_kp = {}
            for ic in range(8):
                for r in range(3):
                    kt = p_k3.tile([128, TC], DT, name="k3", tag="k3")
                    eng = k3q[r] if ic == 0 else nc.sync
                    eng.dma_start(kt[:], K3T[r, 128 * ic:128 * (ic + 1), :])
                    k3[(ic, r)] = kt
                    wt = p_wk.tile([128, DHALF], DT, name="wk", tag="wk")
                    nc.gpsimd.dma_start(
                        wt[:], WEK[1024 * r + 128 * ic:1024 * r + 128 * (ic + 1), :])
                    first = (ic == 0 and r == 0)
                    last = (ic == 7 and r == 2)
                    for m in range(4):
                        for ni, (t0, tw) in enumerate(KPN):
                            if first:
                                ps_kp[(m, ni)] = ps_pool.tile(
                                    [128, 341], f32, name="ps_kp", tag="ps_kp")
                            nc.tensor.matmul(
                                ps_kp[(m, ni)][:, :tw],
                                wt[:, 128 * m:128 * (m + 1)],
                                kt[:, t0:t0 + tw],
                                start=first, stop=last)
            for (m, ni), ps in ps_kp.items():
                t0, tw = KPN[ni]
                # feature rows [0:64] = head 2m, [64:128] = head 2m+1; rows
                # keep their partition placement (no shift needed)
                nc.scalar.copy(kpZ[2 * m][0:64, t0:t0 + tw], ps[0:64, :tw])
                nc.vector.tensor_copy(
                    kpZ[2 * m + 1][64:128, t0:t0 + tw], ps[64:128, :tw])

        # ---- small loads on the scalar queue, issued after the kp phase so
        # they don't compete with K3T/WEK for HBM bandwidth at startup
        mk = []
        for ti in range(8):
            mt = p_mask.tile([128, 512], DT, name="mask", tag="mask")
            nc.scalar.dma_start(mt[:], MASKB[ti])
            mk.append(mt)
        sel = []
        for t4 in range(4):
            s_t = p_small.tile([128, 128], f32r, name="sel", tag="sel", bufs=4)
            nc.scalar.dma_start(s_t[:], SEL4[t4])
            sel.append(s_t)
        wot = {}
        for kk in range(4):
            for nn2 in range(2):
                wt = p_wo.tile([128, 512], DT, name="wo", tag="wo")
                nc.scalar.dma_start(
                    wt[:], WOT[128 * kk:128 * (kk + 1), 512 * nn2:512 * (nn2 + 1)])
                wot[(kk, nn2)] = wt
        vp0 = p_small.tile([1, 520], DT, name="vp0", tag="vp0", bufs=1)
        nc.scalar.dma_start(vp0[:], VP0R[:])
        with nc.allow_non_contiguous_dma(reason="kp col-0 writes, 1KB total"):
            for h in range(H8):
                off = 64 * (h % 2)
                nc.scalar.dma_start(
                    kpZ[h][off:off + 64, 0:1], KP0[64 * h:64 * (h + 1), :])

        # ---- vp_aug fused: vpa[jb][j, 65h+c] = vp[128jb+j, 64h+c], col 65h+64=1
        with tc.tile_pool(name="ps_vp", bufs=6, space="PSUM") as ps_pool:
            ps_vp = {}
            for ic in range(8):
                for r in range(3):
                    wt = p_wv.tile([128, DHALF], DT, name="wv", tag="wv")
                    nc.gpsimd.dma_start(
                        wt[:], WEV[1024 * r + 128 * ic:1024 * r + 128 * (ic + 1), :])
                    first = (ic == 0 and r == 0)
                    last = (ic == 7 and r == 2)
                    for jb in range(NJ):
                        jr = JROWS[jb]
                        if first:
                            ps_vp[jb] = ps_pool.tile(
                                [128, 512], f32, name="ps_vp", tag="ps_vp")
                        nc.tensor.matmul(
                            ps_vp[jb][:jr, :],
                            k3[(ic, r)][:, 128 * jb:128 * jb + jr],
                            wt[:],
                            start=first, stop=last)
            for jb in range(NJ):
                jr = JROWS[jb]
                dst = vpa[jb][:jr].rearrange("p (h c) -> p h c", c=65)
                src = ps_vp[jb][:jr].rearrange("p (h c) -> p h c", c=64)
                if jb % 2 == 0:
                    nc.scalar.copy(dst[:, :, 0:64], src[:])
                else:
                    nc.vector.tensor_copy(dst[:, :, 0:64], src[:])
                nc.vector.tensor_copy(
                    dst[:, :, 64:65],
                    ones_vpa[:jr, :].rearrange("p (h c) -> p h c", c=1))
            # row 0 (key 0) comes from the host: kc[0] = k[0]
            nc.vector.tensor_copy(vpa[0][0:1, :], vp0[:])

        # ---- attention helpers (scores decoupled from o) ----
        ET = {}

        def emit_score(m, h, jc, ps_sc, s_bufs=2):
            hc = h // 2
            jr, c0 = JROWS[jc], C0[(m, jc)]
            ps = ps_sc.tile([128, 512], f32, name="ps_s", tag="ps_s",
                            bufs=s_bufs)
            nc.tensor.matmul(
                ps[:jr, c0:],
                kpZ[h][:, 128 * jc:128 * jc + jr],
                qpt[hc][:, 512 * m + c0:512 * (m + 1)],
                start=True, stop=True)
            et = p_et.tile([128, 512], DT, name="et", tag="et")
            nc.scalar.activation(et[:jr, c0:], ps[:jr, c0:], Exp, scale=SCALE)
            if (m, jc) in RAGGED:
                c1 = RAGGED[(m, jc)]
                ti = RAGGED_LIST.index((m, jc))
                nc.gpsimd.tensor_mul(
                    et[:jr, c0:c1], et[:jr, c0:c1], mk[ti][:jr, c0:c1])
            ET[(m, h, jc)] = et

        # rm zero-padded to K=128 for the full-rate bc matmul (sel rows >= 8
        # are zero, but NaN garbage in rm rows would still poison 0*NaN)
        rm = [p_rm.tile([128, 512], f32r, name="rm", tag="rm") for _ in range(4)]
        for m in range(4):
            nc.vector.memset(rm[m][:].bitcast(f32), 0.0)
        S_m = [p_rm.tile([8, 512], f32, name="sm", tag="sm") for _ in range(4)]

        def emit_o_mm(m, h, jc, po, first, last):
            jr, c0 = JROWS[jc], C0[(m, jc)]
            nc.tensor.matmul(
                po[:65, c0:],
                vpa[jc][:jr, 65 * h:65 * (h + 1)],
                ET[(m, h, jc)][:jr, c0:],
                start=first, stop=last)

        def emit_o_post(m, h, po):
            hc, off = h // 2, (h % 2) * 64
            ss = p_ss.tile([1, 512], f32, name="ss", tag="ss", bufs=2)
            nc.vector.tensor_copy(ss[:], po[64:65, :])
            nc.sync.dma_start(S_m[m][h:h + 1, :], ss[:])
            nc.vector.tensor_copy(o_nt[(hc, m)][off:off + 64, :], po[0:64, :])
            if h == H8 - 1:
                rt = p_ss.tile([8, 512], f32, name="rt", tag="rt", bufs=1)
                nc.vector.reciprocal_approx_fast(out=rt[:], in_=S_m[m][:])
                nc.vector.tensor_copy(rm[m][0:8, :], rt[:])

        def emit_o(m, h, ps_att, o_bufs=2):
            js = JCS[m]
            po = ps_att.tile([128, 512], f32, name="ps_o", tag="ps_o",
                             bufs=o_bufs)
            for jc in js:
                emit_o_mm(m, h, jc, po, jc == js[0], jc == js[-1])
            emit_o_post(m, h, po)

        def emit_bc(m, ps_att):
            for t4 in range(4):
                bc = ps_att.tile([128, 512], f32, name="ps_bc", tag="ps_bc",
                                 bufs=2)
                nc.tensor.matmul(bc[:], sel[t4][:], rm[m][:], start=True,
                                 stop=True)
                for half in range(2):
                    dst = o_nt[(t4, m)][64 * half:64 * half + 64, :]
                    nc.vector.tensor_mul(
                        dst, dst, bc[64 * half:64 * half + 64, :])

        def emit_fin_mq(m, mq, ps_att):
            ob = p_out.tile([128, D], f32, name="outsb", tag="outsb")
            rows = slice(512 * m + 128 * mq, 512 * m + 128 * (mq + 1))
            for nn2 in range(2):
                pf = ps_att.tile([128, 512], f32, name="ps_f", tag="ps_f",
                                 bufs=2)
                for kk in range(4):
                    nc.tensor.matmul(
                        pf[:],
                        o_nt[(kk, m)][:, 128 * mq:128 * (mq + 1)],
                        wot[(kk, nn2)][:],
                        start=(kk == 0), stop=(kk == 3))
                # scalar still runs the exp stream while m=0/1 finalize
                if m < 2 or nn2 == 1:
                    nc.vector.tensor_copy(ob[:, 512 * nn2:512 * (nn2 + 1)], pf[:])
                else:
                    nc.scalar.copy(ob[:, 512 * nn2:512 * (nn2 + 1)], pf[:])
            dq = (nc.gpsimd, nc.sync)[mq % 2]
            dq.dma_start(OUT[rows, :], ob[:])

        # ---- qp passes; scores(m=pass-1) and o(pass-2) interleaved ----
        # PSUM per pass: ps_qp 4 + ps_s 2 + ps_o 2 = 8 banks.
        with tc.tile_pool(name="ps_qp", bufs=8, space="PSUM") as ps_pool:
            for npass in range(4):
                sc_chunks = [[] for _ in range(8)]
                if npass >= 1:
                    msc = npass - 1
                    tiles = [(h, jc) for h in range(H8) for jc in JCS[msc]]
                    sc_chunks = chunk_even(tiles, 8)
                ps_qp = {}
                for kk in range(8):
                    wq_t = p_wq.tile([128, DHALF], DT, name="wq", tag="wq")
                    nc.sync.dma_start(wq_t[:], WQT[128 * kk:128 * (kk + 1), :])
                    qt_t = p_qt.tile([128, 512], DT, name="qt", tag="qt")
                    nc.sync.dma_start(
                        qt_t[:],
                        qT[128 * kk:128 * (kk + 1), 512 * npass:512 * (npass + 1)])
                    # zip qp matmuls with score tiles so the exp stream always
                    # has non-dependent PE work between score matmuls
                    sc_it = list(sc_chunks[kk])
                    for m in range(4):
                        if kk == 0:
                            ps_qp[m] = ps_pool.tile(
                                [128, 512], f32, name="ps_qp", tag="ps_qp",
                                bufs=4)
                        nc.tensor.matmul(
                            ps_qp[m][:],
                            wq_t[:, 128 * m:128 * (m + 1)],
                            qt_t[:],
                            start=(kk == 0), stop=(kk == 7))
                        if sc_it:
                            h, jc = sc_it.pop(0)
                            emit_score(npass - 1, h, jc, ps_pool, s_bufs=3)
                    for (h, jc) in sc_it:
                        emit_score(npass - 1, h, jc, ps_pool, s_bufs=3)
                    if npass >= 2:
                        emit_o(npass - 2, kk, ps_pool, o_bufs=1)
                for m, ps in ps_qp.items():
                    if m % 2 == 0:
                        nc.scalar.copy(
                            qpt[m][:, 512 * npass:512 * (npass + 1)], ps[:])
                    else:
                        nc.vector.tensor_copy(
                            qpt[m][:, 512 * npass:512 * (npass + 1)], ps[:])

        # ---- tail: sc(3) / o(2) / o(3) / finalize pipelined per head ----
        # PSUM: ps_s 2 + ps_o 2 + ps_bc 2 + ps_f 2 = 8 banks.
        with tc.tile_pool(name="ps_att", bufs=2, space="PSUM") as ps_att:
            fin_chunks = []  # deferred finalize chunks for m = 0, 1
            fin_chunks.append(lambda: emit_bc(0, ps_att))
            for mq in range(4):
                fin_chunks.append(lambda m=0, q=mq: emit_fin_mq(m, q, ps_att))
            fin_chunks.append(lambda: emit_bc(1, ps_att))
            for mq in range(4):
                fin_chunks.append(lambda m=1, q=mq: emit_fin_mq(m, q, ps_att))
            ci = 0
            for h in range(H8):
                # zip sc3 / o2 / o3 matmuls so exp latency hides behind
                # independent o-matmuls instead of gating ps_s recycling
                js3, js2 = JCS[3], JCS[2]
                po2 = ps_att.tile([128, 512], f32, name="ps_o", tag="ps_o",
                                  bufs=2)
                emit_score(3, h, js3[0], ps_att)
                emit_score(3, h, js3[1], ps_att)
                for i, jc in enumerate(js2):
                    emit_o_mm(2, h, jc, po2, i == 0, i == len(js2) - 1)
                emit_score(3, h, js3[2], ps_att)
                emit_score(3, h, js3[3], ps_att)
                if h >= 1:
                    po3 = ps_att.tile([128, 512], f32, name="ps_o",
                                      tag="ps_o", bufs=2)
                    for i, jc in enumerate(js3):
                        emit_o_mm(3, h - 1, jc, po3, i == 0, i == 5)
                emit_score(3, h, js3[4], ps_att)
                emit_score(3, h, js3[5], ps_att)
                emit_o_post(2, h, po2)
                if h >= 1:
                    emit_o_post(3, h - 1, po3)
                # 1-2 finalize chunks per slot; bc(1) needs o(1) (done in qp
                # pass 3), all fit behind the sc3/o2/o3 stream
                for _ in range(2 if h >= 4 else 1):
                    if ci < len(fin_chunks):
                        fin_chunks[ci]()
                        ci += 1
            while ci < len(fin_chunks):
                fin_chunks[ci]()
                ci += 1
            emit_bc(2, ps_att)
            emit_o(3, 7, ps_att)
            for mq in range(4):
                emit_fin_mq(2, mq, ps_att)
            # fin(3): all 4 bc tiles live at once (borrow the now-idle ps_s /
            # ps_o rings), normalize per 128-col slice, interleave with the
            # output-projection matmuls to shorten the tail
            bc3 = []
            for t4, tag in enumerate(("ps_bc", "ps_bc", "ps_s", "ps_o")):
                bc = ps_att.tile([128, 512], f32, name="ps_bc3", tag=tag,
                                 bufs=2)
                nc.tensor.matmul(bc[:], sel[t4][:], rm[3][:], start=True,
                                 stop=True)
                bc3.append(bc)
            for mq in range(4):
                for t4 in range(4):
                    for half in range(2):
                        dst = o_nt[(t4, 3)][64 * half:64 * half + 64,
                                            128 * mq:128 * (mq + 1)]
                        nc.vector.tensor_mul(
                            dst, dst,
                            bc3[t4][64 * half:64 * half + 64,
                                    128 * mq:128 * (mq + 1)])
                emit_fin_mq(3, mq, ps_att)

    return nc


def make_maskb():
    import ml_dtypes
    mask = np.zeros((8, 128, 512), dtype=np.float32)
    for t, (m, jc) in enumerate(RAGGED_LIST):
        qq = 512 * m + np.arange(512)[None, :]
        jj = 128 * jc + np.arange(128)[:, None]
        mask[t] = (3 * jj <= qq).astype(np.float32)  # 1.0 where visible
    return mask.astype(ml_dtypes.bfloat16)


def make_sel4():
    sel = np.zeros((4, 128, 128), dtype=np.float32)
    for t in range(4):
        for r in range(128):
            sel[t, 2 * t + r // 64, r] = 1.0
    return sel


def make_k3t(kb):
    """kb: [T, D] f32 for one batch -> K3T [3, D, TC] bf16.
    K3T[r, i, j] = k[3(j-1)+r, i] for j >= 1; column 0 is zero."""
    import ml_dtypes
    out = np.zeros((3, D, TC), dtype=np.float32)
    for r in range(3):
        out[r, :, 1:] = kb[r:r + 3 * (TC - 1):3, :].T
    return out.astype(ml_dtypes.bfloat16)


def prep_inputs(q, k, Wq, Wk, Wv, Wo, conv_w):
    """Returns list of 8 in_maps (core c = 2b + g)."""
    import ml_dtypes
    bf = ml_dtypes.bfloat16
    # Wc3T[r*1024 + i, o] = conv_w[o, i, r]  (so kc[j] = Wc3T.T @ k3(j-1))
    Wc3T = np.ascontiguousarray(
        conv_w.transpose(2, 1, 0).reshape(3 * D, D)).astype(np.float32)
    maskb = make_maskb()
    sel4 = make_sel4()
    halves = []
    for g in range(2):
        sl = slice(DHALF * g, DHALF * (g + 1))
        WEKg = np.ascontiguousarray((Wc3T @ Wk[sl].T)).astype(bf)
        WEVg = np.ascontiguousarray((Wc3T @ Wv[sl].T)).astype(bf)
        halves.append((sl, WEKg, WEVg))
    k3ts = [make_k3t(k[b]) for b in range(B)]
    in_maps = []
    for c in range(8):
        b, g = c // 2, c % 2
        sl, WEKg, WEVg = halves[g]
        kp0 = (Wk[sl] @ k[b, 0]).astype(np.float32).reshape(DHALF, 1)
        vp0 = (Wv[sl] @ k[b, 0]).astype(np.float32)
        vp0r = np.zeros((8, 65), np.float32)
        vp0r[:, :64] = vp0.reshape(8, 64)
        vp0r[:, 64] = 1.0
        in_maps.append({
            "qT": np.ascontiguousarray(q[b].T).astype(bf),
            "K3T": k3ts[b],
            "WEK": WEKg,
            "WEV": WEVg,
            "WQT": np.ascontiguousarray(Wq[sl, :].T).astype(bf),
            "WOT": np.ascontiguousarray(Wo[:, sl].T).astype(bf),
            "MASKB": maskb,
            "SEL4": sel4,
            "KP0": kp0.astype(bf),
            "VP0R": vp0r.reshape(1, 520).astype(bf),
        })
    return in_maps


def postprocess(results, bo):
    out = np.zeros((B, T, D), dtype=np.float32)
    for b in range(B):
        out[b] = (np.asarray(results[2 * b]["out_p"], dtype=np.float32)
                  + np.asarray(results[2 * b + 1]["out_p"], dtype=np.float32)
                  + bo[None, :])
    return out


_CACHED_NC = None


def kernel(q, k, v, Wq, Wk, Wv, Wo, bo, conv_w):
    """Full-input entry point. v is unused by the reference computation
    (V is replaced by the conv-compressed K)."""
    global _CACHED_NC
    from concourse.bass_utils import run_bass_kernel_spmd

    q = np.asarray(q, dtype=np.float32)
    k = np.asarray(k, dtype=np.float32)
    Wq = np.asarray(Wq, dtype=np.float32)
    Wk = np.asarray(Wk, dtype=np.float32)
    Wv = np.asarray(Wv, dtype=np.float32)
    Wo = np.asarray(Wo, dtype=np.float32)
    bo = np.asarray(bo, dtype=np.float32)
    conv_w = np.asarray(conv_w, dtype=np.float32)

    in_maps = prep_inputs(q, k, Wq, Wk, Wv, Wo, conv_w)
    if _CACHED_NC is None:
        nc = build_nc()
        nc.finalize()
        _CACHED_NC = nc
    res = run_bass_kernel_spmd(_CACHED_NC, in_maps, list(range(8)))
    return postprocess(res.results, bo)


# revision 63
# speedup vs baseline: 1.0283x; 1.0115x over previous
"""Sparse (conv-compressed) multi-head attention on 8 Trainium2 NeuronCores.

Entry point: kernel(**inputs) -> np.ndarray [4, 2048, 1024] float32.

Sharding: core c = 2*b + g  (b = batch 0..3, g = head-half 0..1).
Each core: batch b, heads [8g, 8g+8), all 2048 queries.
Final projection produces a partial (dv-half contraction); host sums pairs + bias.

v3 design notes:
- The strided conv that builds the compressed keys kc is FUSED into the
  Wk/Wv projections on the host:  kp = (Wk_hh @ Wc3) @ k3,  vp likewise,
  where k3(t) = concat(k[3t], k[3t+1], k[3t+2]).  No conv intermediate,
  no cross-core collective.  Column j=0 (kc[0] = k[0]) comes from the host.
- The host ships kT pre-decimated as K3T[r, i, j] = k[3(j-1)+r, i] so every
  matmul operand is contiguous (strided SBUF reads cost ~1.5x on the PE).
- Score matmuls + exp are decoupled from the attention-value matmuls: scores
  stream into the qp projection passes so the scalar engine's exp pipeline
  (the second-longest serial resource) starts ~60us earlier and never paces
  the PE.  o/normalize/output-projection work is interleaved per q-tile at
  the tail.
- Causal mask applied as a 0/1 bf16 multiply on eT after exp (gpsimd).
"""
import sys
sys.path.insert(0, '/opt/trn_rl_repo')
import numpy as np
import concourse.bass as bass
import concourse.bacc as bacc
import concourse.mybir as mybir
from concourse import tile
from contextlib import ExitStack

f32 = mybir.dt.float32
f32r = mybir.dt.float32r
bf16 = mybir.dt.bfloat16
DT = bf16
Exp = mybir.ActivationFunctionType.Exp

B, T, D, H = 4, 2048, 1024, 16
DH = 64
TC = 683          # compressed keys: 1 + 682
DHALF = D // 2    # per-core head-half width
H8 = H // 2       # heads per core
SCALE = DH ** -0.5  # 0.125

# kp n-splits over keys [1, 683)
KPN = [(1, 341), (342, 341)]

# attention q-tiles (4 x 512) and j-chunks (6 x 128, last = 43 rows)
NJ = 6
JROWS = [128, 128, 128, 128, 128, TC - 5 * 128]  # last = 43
JCS = {m: [jc for jc in range(NJ) if 384 * jc < 512 * (m + 1)] for m in range(4)}
C0 = {(m, jc): max(0, 384 * jc - 512 * m) for m in range(4) for jc in JCS[m]}
# ragged tiles (m, jc) -> first fully-visible column c1 (cols [c0, c1) get mask)
RAGGED = {}
for m in range(4):
    for jc in JCS[m]:
        if not (384 * jc + 381 <= 512 * m):
            RAGGED[(m, jc)] = min(512, 384 * jc + 381 - 512 * m)
RAGGED_LIST = sorted(RAGGED.keys())  # 8 tiles
assert len(RAGGED_LIST) == 8


def chunk_even(lst, n):
    """Split lst into n chunks with sizes as even as possible."""
    k, r = divmod(len(lst), n)
    out, i = [], 0
    for j in range(n):
        sz = k + (1 if j < r else 0)
        out.append(lst[i:i + sz])
        i += sz
    return out


def build_nc():
    nc = bacc.Bacc(None, target_bir_lowering=False, debug=False)

    qT = nc.dram_tensor("qT", [D, T], DT, kind="ExternalInput")
    K3T = nc.dram_tensor("K3T", [3, D, TC], DT, kind="ExternalInput")
    WEK = nc.dram_tensor("WEK", [3 * D, DHALF], DT, kind="ExternalInput")
    WEV = nc.dram_tensor("WEV", [3 * D, DHALF], DT, kind="ExternalInput")
    WQT = nc.dram_tensor("WQT", [D, DHALF], DT, kind="ExternalInput")
    WOT = nc.dram_tensor("WOT", [DHALF, D], DT, kind="ExternalInput")
    MASKB = nc.dram_tensor("MASKB", [8, 128, 512], DT, kind="ExternalInput")
    SEL4 = nc.dram_tensor("SEL4", [4, 128, 128], f32r, kind="ExternalInput")
    KP0 = nc.dram_tensor("KP0", [DHALF, 1], DT, kind="ExternalInput")
    VP0R = nc.dram_tensor("VP0R", [1, 520], DT, kind="ExternalInput")
    OUT = nc.dram_tensor("out_p", [T, D], f32, kind="ExternalOutput")

    with tile.TileContext(nc) as tc, ExitStack() as st:
        st.enter_context(nc.allow_low_precision("bf16 matmuls, f32r recip bcast"))
        pool = lambda **kw: st.enter_context(tc.tile_pool(**kw))
        p_k3 = pool(name="k3", bufs=24)       # K3T chunks [128, 683]
        p_wk = pool(name="wk", bufs=6)        # WEK chunks [128, 512]
        p_wv = pool(name="wv", bufs=6)        # WEV chunks [128, 512]
        p_wq = pool(name="wq", bufs=8)        # WQT chunks [128, 512]
        p_qt = pool(name="qt", bufs=8)        # qT chunks [128, 512]
        p_kpt = pool(name="kpt", bufs=8)      # kp^T per head, K zero-padded
        p_vpa = pool(name="vpa", bufs=6)      # vp_aug [128, 520]
        p_qpt = pool(name="qpt", bufs=4)      # qp^T [128, 2048]
        p_mask = pool(name="mask", bufs=8)    # 0/1 bf16 masks [128, 512]
        p_et = pool(name="et", bufs=42)       # exp(scores) [128, 512] bf16
        p_ont = pool(name="ont", bufs=16)     # unnormalized head outs [128, 512]
        p_wo = pool(name="wo", bufs=8)        # WoT resident [128, 512]
        p_out = pool(name="outsb", bufs=3)    # out staging [128, 1024] f32
        p_rm = pool(name="rm", bufs=4)        # per-m recip denominators [8, 512]
        p_ss = pool(name="ss", bufs=8)        # denom staging rows [1, 512]
        p_small = pool(name="small", bufs=6)

        # kpZ[h]: rows 0-63 = head h's kp dims, rows 64-127 = ZERO.  Scores
        # then contract K=128 (full-rate: K=64 matmuls stream at half speed);
        # the zero rows null out the other head's qp rows in the rhs.
        # head h lives in qpt partition rows [64*(h%2), +64); kpZ mirrors that
        # row placement and zeroes the other head's rows
        kpZ = [p_kpt.tile([128, TC], DT, name="kpt", tag="kpt") for _ in range(H8)]
        for h in range(H8):
            z0 = 64 * ((h + 1) % 2)
            nc.vector.memset(kpZ[h][z0:z0 + 64, :], 0.0)
        vpa = [p_vpa.tile([128, 520], DT, name="vpa", tag="vpa") for _ in range(NJ)]
        qpt = [p_qpt.tile([128, T], DT, name="qpt", tag="qpt") for _ in range(4)]
        o_nt = {(kk, m): p_ont.tile([128, 512], DT, name="ont", tag="ont")
                for kk in range(4) for m in range(4)}
        ones_vpa = p_small.tile([128, 8], f32, name="ones_vpa", tag="ones_vpa",
                                bufs=1)
        nc.vector.memset(ones_vpa[:], 1.0)

        # ---- kp^T fused: kpt[m][:, j] = (Wk_hh @ Wc3) @ k3(j-1), j in [1, 683)
        k3 = {}
        k3q = [nc.sync, nc.scalar, nc.gpsimd]  # parallel issue for fast start
        with tc.tile_pool(name="ps_kp", bufs=8, space="PSUM") as ps_pool:
            ps_kp = {}
            for ic in range(8):
                for r in range(3):
                    kt = p_k3.tile([128, TC], DT, name="k3", tag="k3")
                    eng = k3q[r] if ic == 0 else nc.sync
                    eng.dma_start(kt[:], K3T[r, 128 * ic:128 * (ic + 1), :])
                    k3[(ic, r)] = kt
                    wt = p_wk.tile([128, DHALF], DT, name="wk", tag="wk")
                    nc.gpsimd.dma_start(
                        wt[:], WEK[1024 * r + 128 * ic:1024 * r + 128 * (ic + 1), :])
                    first = (ic == 0 and r == 0)
                    last = (ic == 7 and r == 2)
                    for m in range(4):
                        for ni, (t0, tw) in enumerate(KPN):
                            if first:
                                ps_kp[(m, ni)] = ps_pool.tile(
                                    [128, 341], f32, name="ps_kp", tag="ps_kp")
                            nc.tensor.matmul(
                                ps_kp[(m, ni)][:, :tw],
                                wt[:, 128 * m:128 * (m + 1)],
                                kt[:, t0:t0 + tw],
                                start=first, stop=last)
            for (m, ni), ps in ps_kp.items():
                t0, tw = KPN[ni]
                # feature rows [0:64] = head 2m, [64:128] = head 2m+1; rows
                # keep their partition placement (no shift needed)
                nc.scalar.copy(kpZ[2 * m][0:64, t0:t0 + tw], ps[0:64, :tw])
                nc.vector.tensor_copy(
                    kpZ[2 * m + 1][64:128, t0:t0 + tw], ps[64:128, :tw])

        # ---- small loads on the scalar queue, issued after the kp phase so
        # they don't compete with K3T/WEK for HBM bandwidth at startup
        mk = []
        for ti in range(8):
            mt = p_mask.tile([128, 512], DT, name="mask", tag="mask")
            nc.scalar.dma_start(mt[:], MASKB[ti])
            mk.append(mt)
        sel = []
        for t4 in range(4):
            s_t = p_small.tile([128, 128], f32r, name="sel", tag="sel", bufs=4)
            nc.scalar.dma_start(s_t[:], SEL4[t4])
            sel.append(s_t)
        wot = {}
        for kk in range(4):
            for nn2 in range(2):
                wt = p_wo.tile([128, 512], DT, name="wo", tag="wo")
                nc.scalar.dma_start(
                    wt[:], WOT[128 * kk:128 * (kk + 1), 512 * nn2:512 * (nn2 + 1)])
                wot[(kk, nn2)] = wt
        vp0 = p_small.tile([1, 520], DT, name="vp0", tag="vp0", bufs=1)
        nc.scalar.dma_start(vp0[:], VP0R[:])
        with nc.allow_non_contiguous_dma(reason="kp col-0 writes, 1KB total"):
            for h in range(H8):
                off = 64 * (h % 2)
                nc.scalar.dma_start(
                    kpZ[h][off:off + 64, 0:1], KP0[64 * h:64 * (h + 1), :])

        # ---- vp_aug fused: vpa[jb][j, 65h+c] = vp[128jb+j, 64h+c], col 65h+64=1
        with tc.tile_pool(name="ps_vp", bufs=6, space="PSUM") as ps_pool:
            ps_vp = {}
            for ic in range(8):
                for r in range(3):
                    wt = p_wv.tile([128, DHALF], DT, name="wv", tag="wv")
                    nc.gpsimd.dma_start(
                        wt[:], WEV[1024 * r + 128 * ic:1024 * r + 128 * (ic + 1), :])
                    first = (ic == 0 and r == 0)
                    last = (ic == 7 and r == 2)
                    for jb in range(NJ):
                        jr = JROWS[jb]
                        if first:
                            ps_vp[jb] = ps_pool.tile(
                                [128, 512], f32, name="ps_vp", tag="ps_vp")
                        nc.tensor.matmul(
                            ps_vp[jb][:jr, :],
                            k3[(ic, r)][:, 128 * jb:128 * jb + jr],
                            wt[:],
                            start=first, stop=last)
            for jb in range(NJ):
                jr = JROWS[jb]
                dst = vpa[jb][:jr].rearrange("p (h c) -> p h c", c=65)
                src = ps_vp[jb][:jr].rearrange("p (h c) -> p h c", c=64)
                if jb % 2 == 0:
                    nc.scalar.copy(dst[:, :, 0:64], src[:])
                else:
                    nc.vector.tensor_copy(dst[:, :, 0:64], src[:])
                nc.vector.tensor_copy(
                    dst[:, :, 64:65],
                    ones_vpa[:jr, :].rearrange("p (h c) -> p h c", c=1))
            # row 0 (key 0) comes from the host: kc[0] = k[0]
            nc.vector.tensor_copy(vpa[0][0:1, :], vp0[:])

        # ---- attention helpers (scores decoupled from o) ----
        ET = {}

        def emit_score(m, h, jc, ps_sc, s_bufs=2):
            hc = h // 2
            jr, c0 = JROWS[jc], C0[(m, jc)]
            ps = ps_sc.tile([128, 512], f32, name="ps_s", tag="ps_s",
                            bufs=s_bufs)
            nc.tensor.matmul(
                ps[:jr, c0:],
                kpZ[h][:, 128 * jc:128 * jc + jr],
                qpt[hc][:, 512 * m + c0:512 * (m + 1)],
                start=True, stop=True)
            et = p_et.tile([128, 512], DT, name="et", tag="et")
            nc.scalar.activation(et[:jr, c0:], ps[:jr, c0:], Exp, scale=SCALE)
            if (m, jc) in RAGGED:
                c1 = RAGGED[(m, jc)]
                ti = RAGGED_LIST.index((m, jc))
                nc.gpsimd.tensor_mul(
                    et[:jr, c0:c1], et[:jr, c0:c1], mk[ti][:jr, c0:c1])
            ET[(m, h, jc)] = et

        # rm zero-padded to K=128 for the full-rate bc matmul (sel rows >= 8
        # are zero, but NaN garbage in rm rows would still poison 0*NaN)
        rm = [p_rm.tile([128, 512], f32r, name="rm", tag="rm") for _ in range(4)]
        for m in range(4):
            nc.vector.memset(rm[m][:].bitcast(f32), 0.0)
        S_m = [p_rm.tile([8, 512], f32, name="sm", tag="sm") for _ in range(4)]

        def emit_o_mm(m, h, jc, po, first, last):
            jr, c0 = JROWS[jc], C0[(m, jc)]
            nc.tensor.matmul(
                po[:65, c0:],
                vpa[jc][:jr, 65 * h:65 * (h + 1)],
                ET[(m, h, jc)][:jr, c0:],
                start=first, stop=last)

        def emit_o_post(m, h, po):
            hc, off = h // 2, (h % 2) * 64
            ss = p_ss.tile([1, 512], f32, name="ss", tag="ss", bufs=2)
            nc.vector.tensor_copy(ss[:], po[64:65, :])
            nc.sync.dma_start(S_m[m][h:h + 1, :], ss[:])
            nc.vector.tensor_copy(o_nt[(hc, m)][off:off + 64, :], po[0:64, :])
            if h == H8 - 1:
                rt = p_ss.tile([8, 512], f32, name="rt", tag="rt", bufs=1)
                nc.vector.reciprocal_approx_fast(out=rt[:], in_=S_m[m][:])
                nc.vector.tensor_copy(rm[m][0:8, :], rt[:])

        def emit_o(m, h, ps_att, o_bufs=2):
            js = JCS[m]
            po = ps_att.tile([128, 512], f32, name="ps_o", tag="ps_o",
                             bufs=o_bufs)
            for jc in js:
                emit_o_mm(m, h, jc, po, jc == js[0], jc == js[-1])
            emit_o_post(m, h, po)

        def emit_bc(m, ps_att):
            for t4 in range(4):
                bc = ps_att.tile([128, 512], f32, name="ps_bc", tag="ps_bc",
                                 bufs=2)
                nc.tensor.matmul(bc[:], sel[t4][:], rm[m][:], start=True,
                                 stop=True)
                for half in range(2):
                    dst = o_nt[(t4, m)][64 * half:64 * half + 64, :]
                    nc.vector.tensor_mul(
                        dst, dst, bc[64 * half:64 * half + 64, :])

        def emit_fin_mq(m, mq, ps_att):
            ob = p_out.tile([128, D], f32, name="outsb", tag="outsb")
            rows = slice(512 * m + 128 * mq, 512 * m + 128 * (mq + 1))
            for nn2 in range(2):
                pf = ps_att.tile([128, 512], f32, name="ps_f", tag="ps_f",
                                 bufs=2)
                for kk in range(4):
                    nc.tensor.matmul(
                        pf[:],
                        o_nt[(kk, m)][:, 128 * mq:128 * (mq + 1)],
                        wot[(kk, nn2)][:],
                        start=(kk == 0), stop=(kk == 3))
                # scalar still runs the exp stream while m=0/1 finalize
                if m < 2:
                    nc.vector.tensor_copy(ob[:, 512 * nn2:512 * (nn2 + 1)], pf[:])
                else:
                    nc.scalar.copy(ob[:, 512 * nn2:512 * (nn2 + 1)], pf[:])
            dq = (nc.gpsimd, nc.sync)[mq % 2]
            dq.dma_start(OUT[rows, :], ob[:])

        # ---- qp passes; scores(m=pass-1) and o(pass-2) interleaved ----
        # PSUM per pass: ps_qp 4 + ps_s 2 + ps_o 2 = 8 banks.
        with tc.tile_pool(name="ps_qp", bufs=8, space="PSUM") as ps_pool:
            for npass in range(4):
                sc_chunks = [[] for _ in range(8)]
                if npass >= 1:
                    msc = npass - 1
                    tiles = [(h, jc) for h in range(H8) for jc in JCS[msc]]
                    sc_chunks = chunk_even(tiles, 8)
                ps_qp = {}
                for kk in range(8):
                    wq_t = p_wq.tile([128, DHALF], DT, name="wq", tag="wq")
                    nc.sync.dma_start(wq_t[:], WQT[128 * kk:128 * (kk + 1), :])
                    qt_t = p_qt.tile([128, 512], DT, name="qt", tag="qt")
                    nc.sync.dma_start(
                        qt_t[:],
                        qT[128 * kk:128 * (kk + 1), 512 * npass:512 * (npass + 1)])
                    # zip qp matmuls with score tiles so the exp stream always
                    # has non-dependent PE work between score matmuls
                    sc_it = list(sc_chunks[kk])
                    for m in range(4):
                        if kk == 0:
                            ps_qp[m] = ps_pool.tile(
                                [128, 512], f32, name="ps_qp", tag="ps_qp",
                                bufs=4)
                        nc.tensor.matmul(
                            ps_qp[m][:],
                            wq_t[:, 128 * m:128 * (m + 1)],
                            qt_t[:],
                            start=(kk == 0), stop=(kk == 7))
                        if sc_it:
                            h, jc = sc_it.pop(0)
                            emit_score(npass - 1, h, jc, ps_pool)
                    for (h, jc) in sc_it:
                        emit_score(npass - 1, h, jc, ps_pool)
                    if npass >= 2:
                        emit_o(npass - 2, kk, ps_pool)
                for m, ps in ps_qp.items():
                    if m % 2 == 0:
                        nc.scalar.copy(
                            qpt[m][:, 512 * npass:512 * (npass + 1)], ps[:])
                    else:
                        nc.vector.tensor_copy(
                            qpt[m][:, 512 * npass:512 * (npass + 1)], ps[:])

        # ---- tail: sc(3) / o(2) / o(3) / finalize pipelined per head ----
        # PSUM: ps_s 2 + ps_o 2 + ps_bc 2 + ps_f 2 = 8 banks.
        with tc.tile_pool(name="ps_att", bufs=2, space="PSUM") as ps_att:
            fin_chunks = []  # deferred finalize chunks for m = 0, 1
            fin_chunks.append(lambda: emit_bc(0, ps_att))
            for mq in range(4):
                fin_chunks.append(lambda m=0, q=mq: emit_fin_mq(m, q, ps_att))
            fin_chunks.append(lambda: emit_bc(1, ps_att))
            for mq in range(4):
                fin_chunks.append(lambda m=1, q=mq: emit_fin_mq(m, q, ps_att))
            ci = 0
            for h in range(H8):
                # zip sc3 / o2 / o3 matmuls so exp latency hides behind
                # independent o-matmuls instead of gating ps_s recycling
                js3, js2 = JCS[3], JCS[2]
                po2 = ps_att.tile([128, 512], f32, name="ps_o", tag="ps_o",
                                  bufs=2)
                emit_score(3, h, js3[0], ps_att)
                emit_score(3, h, js3[1], ps_att)
                for i, jc in enumerate(js2):
                    emit_o_mm(2, h, jc, po2, i == 0, i == len(js2) - 1)
                emit_score(3, h, js3[2], ps_att)
                emit_score(3, h, js3[3], ps_att)
                if h >= 1:
                    po3 = ps_att.tile([128, 512], f32, name="ps_o",
                                      tag="ps_o", bufs=2)
                    for i, jc in enumerate(js3):
                        emit_o_mm(3, h - 1, jc, po3, i == 0, i == 5)
                emit_score(3, h, js3[4], ps_att)
                emit_score(3, h, js3[5], ps_att)
                emit_o_post(2, h, po2)
                if h >= 1:
                    emit_o_post(3, h - 1, po3)
                # 1-2 finalize chunks per slot; bc(1) needs o(1) (done in qp
                # pass 3), all fit behind the sc3/o2/o3 stream
                for _ in range(2 if h >= 4 else 1):
                    if ci < len(fin_chunks):
                        fin_chunks[ci]()
                        ci += 1
            while ci < len(fin_chunks):
                fin_chunks[ci]()
                ci += 1
            emit_bc(2, ps_att)
            emit_o(3, 7, ps_att)
            for mq in range(4):
                emit_fin_mq(2, mq, ps_att)
            # fin(3): all 4 bc tiles live at once (borrow the now-idle ps_s /
            # ps_o rings), normalize per 128-col slice, interleave with the
            # output-projection matmuls to shorten the tail
            bc3 = []
            for t4, tag in enumerate(("ps_bc", "ps_bc", "ps_s", "ps_o")):
                bc = ps_att.tile([128, 512], f32, name="ps_bc3", tag=tag,
                                 bufs=2)
                nc.tensor.matmul(bc[:], sel[t4][:], rm[3][:], start=True,
                                 stop=True)
                bc3.append(bc)
            for mq in range(4):
                for t4 in range(4):
                    for half in range(2):
                        dst = o_nt[(t4, 3)][64 * half:64 * half + 64,
                                            128 * mq:128 * (mq + 1)]
                        nc.vector.tensor_mul(
                            dst, dst,
                            bc3[t4][64 * half:64 * half + 64,
                                    128 * mq:128 * (mq + 1)])
                emit_fin_mq(3, mq, ps_att)

    return nc


def make_maskb():
    import ml_dtypes
    mask = np.zeros((8, 128, 512), dtype=np.float32)
    for t, (m, jc) in enumerate(RAGGED_LIST):
        qq = 512 * m + np.arange(512)[None, :]
        jj = 128 * jc + np.arange(128)[:, None]
        mask[t] = (3 * jj <= qq).astype(np.float32)  # 1.0 where visible
    return mask.astype(ml_dtypes.bfloat16)


def make_sel4():
    sel = np.zeros((4, 128, 128), dtype=np.float32)
    for t in range(4):
        for r in range(128):
            sel[t, 2 * t + r // 64, r] = 1.0
    return sel


def make_k3t(kb):
    """kb: [T, D] f32 for one batch -> K3T [3, D, TC] bf16.
    K3T[r, i, j] = k[3(j-1)+r, i] for j >= 1; column 0 is zero."""
    import ml_dtypes
    out = np.zeros((3, D, TC), dtype=np.float32)
    for r in range(3):
        out[r, :, 1:] = kb[r:r + 3 * (TC - 1):3, :].T
    return out.astype(ml_dtypes.bfloat16)


def prep_inputs(q, k, Wq, Wk, Wv, Wo, conv_w):
    """Returns list of 8 in_maps (core c = 2b + g)."""
    import ml_dtypes
    bf = ml_dtypes.bfloat16
    # Wc3T[r*1024 + i, o] = conv_w[o, i, r]  (so kc[j] = Wc3T.T @ k3(j-1))
    Wc3T = np.ascontiguousarray(
        conv_w.transpose(2, 1, 0).reshape(3 * D, D)).astype(np.float32)
    maskb = make_maskb()
    sel4 = make_sel4()
    halves = []
    for g in range(2):
        sl = slice(DHALF * g, DHALF * (g + 1))
        WEKg = np.ascontiguousarray((Wc3T @ Wk[sl].T)).astype(bf)
        WEVg = np.ascontiguousarray((Wc3T @ Wv[sl].T)).astype(bf)
        halves.append((sl, WEKg, WEVg))
    k3ts = [make_k3t(k[b]) for b in range(B)]
    in_maps = []
    for c in range(8):
        b, g = c // 2, c % 2
        sl, WEKg, WEVg = halves[g]
        kp0 = (Wk[sl] @ k[b, 0]).astype(np.float32).reshape(DHALF, 1)
        vp0 = (Wv[sl] @ k[b, 0]).astype(np.float32)
        vp0r = np.zeros((8, 65), np.float32)
        vp0r[:, :64] = vp0.reshape(8, 64)
        vp0r[:, 64] = 1.0
        in_maps.append({
            "qT": np.ascontiguousarray(q[b].T).astype(bf),
            "K3T": k3ts[b],
            "WEK": WEKg,
            "WEV": WEVg,
            "WQT": np.ascontiguousarray(Wq[sl, :].T).astype(bf),
            "WOT": np.ascontiguousarray(Wo[:, sl].T).astype(bf),
            "MASKB": maskb,
            "SEL4": sel4,
            "KP0": kp0.astype(bf),
            "VP0R": vp0r.reshape(1, 520).astype(bf),
        })
    return in_maps


def postprocess(results, bo):
    out = np.zeros((B, T, D), dtype=np.float32)
    for b in range(B):
        out[b] = (np.asarray(results[2 * b]["out_p"], dtype=np.float32)
                  + np.asarray(results[2 * b + 1]["out_p"], dtype=np.float32)
                  + bo[None, :])
    return out


_CACHED_NC = None


def kernel(q, k, v, Wq, Wk, Wv, Wo, bo, conv_w):
    """Full-input entry point. v is unused by the reference computation
    (V is replaced by the conv-compressed K)."""
    global _CACHED_NC
    from concourse.bass_utils import run_bass_kernel_spmd

    q = np.asarray(q, dtype=np.float32)
    k = np.asarray(k, dtype=np.float32)
    Wq = np.asarray(Wq, dtype=np.float32)
    Wk = np.asarray(Wk, dtype=np.float32)
    Wv = np.asarray(Wv, dtype=np.float32)
    Wo = np.asarray(Wo, dtype=np.float32)
    bo = np.asarray(bo, dtype=np.float32)
    conv_w = np.asarray(conv_w, dtype=np.float32)

    in_maps = prep_inputs(q, k, Wq, Wk, Wv, Wo, conv_w)
    if _CACHED_NC is None:
        nc = build_nc()
        nc.finalize()
        _CACHED_NC = nc
    res = run_bass_kernel_spmd(_CACHED_NC, in_maps, list(range(8)))
    return postprocess(res.results, bo)


# revision 64
# speedup vs baseline: 1.0418x; 1.0131x over previous
"""Sparse (conv-compressed) multi-head attention on 8 Trainium2 NeuronCores.

Entry point: kernel(**inputs) -> np.ndarray [4, 2048, 1024] float32.

Sharding: core c = 2*b + g  (b = batch 0..3, g = head-half 0..1).
Each core: batch b, heads [8g, 8g+8), all 2048 queries.
Final projection produces a partial (dv-half contraction); host sums pairs + bias.

v3 design notes:
- The strided conv that builds the compressed keys kc is FUSED into the
  Wk/Wv projections on the host:  kp = (Wk_hh @ Wc3) @ k3,  vp likewise,
  where k3(t) = concat(k[3t], k[3t+1], k[3t+2]).  No conv intermediate,
  no cross-core collective.  Column j=0 (kc[0] = k[0]) comes from the host.
- The host ships kT pre-decimated as K3T[r, i, j] = k[3(j-1)+r, i] so every
  matmul operand is contiguous (strided SBUF reads cost ~1.5x on the PE).
- Score matmuls + exp are decoupled from the attention-value matmuls: scores
  stream into the qp projection passes so the scalar engine's exp pipeline
  (the second-longest serial resource) starts ~60us earlier and never paces
  the PE.  o/normalize/output-projection work is interleaved per q-tile at
  the tail.
- Causal mask applied as a 0/1 bf16 multiply on eT after exp (gpsimd).
"""
import sys
sys.path.insert(0, '/opt/trn_rl_repo')
import numpy as np
import concourse.bass as bass
import concourse.bacc as bacc
import concourse.mybir as mybir
from concourse import tile
from contextlib import ExitStack

f32 = mybir.dt.float32
f32r = mybir.dt.float32r
bf16 = mybir.dt.bfloat16
DT = bf16
Exp = mybir.ActivationFunctionType.Exp

B, T, D, H = 4, 2048, 1024, 16
DH = 64
TC = 683          # compressed keys: 1 + 682
DHALF = D // 2    # per-core head-half width
H8 = H // 2       # heads per core
SCALE = DH ** -0.5  # 0.125

# kp n-splits over keys [1, 683)
KPN = [(1, 341), (342, 341)]

# attention q-tiles (4 x 512) and j-chunks (6 x 128, last = 43 rows)
NJ = 6
JROWS = [128, 128, 128, 128, 128, TC - 5 * 128]  # last = 43
JCS = {m: [jc for jc in range(NJ) if 384 * jc < 512 * (m + 1)] for m in range(4)}
C0 = {(m, jc): max(0, 384 * jc - 512 * m) for m in range(4) for jc in JCS[m]}
# ragged tiles (m, jc) -> first fully-visible column c1 (cols [c0, c1) get mask)
RAGGED = {}
for m in range(4):
    for jc in JCS[m]:
        if not (384 * jc + 381 <= 512 * m):
            RAGGED[(m, jc)] = min(512, 384 * jc + 381 - 512 * m)
RAGGED_LIST = sorted(RAGGED.keys())  # 8 tiles
assert len(RAGGED_LIST) == 8


def chunk_even(lst, n):
    """Split lst into n chunks with sizes as even as possible."""
    k, r = divmod(len(lst), n)
    out, i = [], 0
    for j in range(n):
        sz = k + (1 if j < r else 0)
        out.append(lst[i:i + sz])
        i += sz
    return out


def build_nc():
    nc = bacc.Bacc(None, target_bir_lowering=False, debug=False)

    qT = nc.dram_tensor("qT", [D, T], DT, kind="ExternalInput")
    K3T = nc.dram_tensor("K3T", [3, D, TC], DT, kind="ExternalInput")
    WEK = nc.dram_tensor("WEK", [3 * D, DHALF], DT, kind="ExternalInput")
    WEV = nc.dram_tensor("WEV", [3 * D, DHALF], DT, kind="ExternalInput")
    WQT = nc.dram_tensor("WQT", [D, DHALF], DT, kind="ExternalInput")
    WOT = nc.dram_tensor("WOT", [DHALF, D], DT, kind="ExternalInput")
    MASKB = nc.dram_tensor("MASKB", [8, 128, 512], DT, kind="ExternalInput")
    SEL4 = nc.dram_tensor("SEL4", [4, 128, 128], f32r, kind="ExternalInput")
    KP0 = nc.dram_tensor("KP0", [DHALF, 1], DT, kind="ExternalInput")
    VP0R = nc.dram_tensor("VP0R", [1, 520], DT, kind="ExternalInput")
    OUT = nc.dram_tensor("out_p", [T, D], f32, kind="ExternalOutput")

    with tile.TileContext(nc) as tc, ExitStack() as st:
        st.enter_context(nc.allow_low_precision("bf16 matmuls, f32r recip bcast"))
        pool = lambda **kw: st.enter_context(tc.tile_pool(**kw))
        p_k3 = pool(name="k3", bufs=24)       # K3T chunks [128, 683]
        p_wk = pool(name="wk", bufs=6)        # WEK chunks [128, 512]
        p_wv = pool(name="wv", bufs=6)        # WEV chunks [128, 512]
        p_wq = pool(name="wq", bufs=8)        # WQT chunks [128, 512]
        p_qt = pool(name="qt", bufs=8)        # qT chunks [128, 512]
        p_kpt = pool(name="kpt", bufs=8)      # kp^T per head, K zero-padded
        p_vpa = pool(name="vpa", bufs=6)      # vp_aug [128, 520]
        p_qpt = pool(name="qpt", bufs=4)      # qp^T [128, 2048]
        p_mask = pool(name="mask", bufs=8)    # 0/1 bf16 masks [128, 512]
        p_et = pool(name="et", bufs=42)       # exp(scores) [128, 512] bf16
        p_ont = pool(name="ont", bufs=16)     # unnormalized head outs [128, 512]
        p_wo = pool(name="wo", bufs=8)        # WoT resident [128, 512]
        p_out = pool(name="outsb", bufs=3)    # out staging [128, 1024] f32
        p_rm = pool(name="rm", bufs=4)        # per-m recip denominators [8, 512]
        p_ss = pool(name="ss", bufs=8)        # denom staging rows [1, 512]
        p_small = pool(name="small", bufs=6)

        # kpZ[h]: rows 0-63 = head h's kp dims, rows 64-127 = ZERO.  Scores
        # then contract K=128 (full-rate: K=64 matmuls stream at half speed);
        # the zero rows null out the other head's qp rows in the rhs.
        # head h lives in qpt partition rows [64*(h%2), +64); kpZ mirrors that
        # row placement and zeroes the other head's rows
        kpZ = [p_kpt.tile([128, TC], DT, name="kpt", tag="kpt") for _ in range(H8)]
        for h in range(H8):
            z0 = 64 * ((h + 1) % 2)
            nc.vector.memset(kpZ[h][z0:z0 + 64, :], 0.0)
        vpa = [p_vpa.tile([128, 520], DT, name="vpa", tag="vpa") for _ in range(NJ)]
        qpt = [p_qpt.tile([128, T], DT, name="qpt", tag="qpt") for _ in range(4)]
        o_nt = {(kk, m): p_ont.tile([128, 512], DT, name="ont", tag="ont")
                for kk in range(4) for m in range(4)}
        ones_vpa = p_small.tile([128, 8], f32, name="ones_vpa", tag="ones_vpa",
                                bufs=1)
        nc.vector.memset(ones_vpa[:], 1.0)

        # ---- kp^T fused: kpt[m][:, j] = (Wk_hh @ Wc3) @ k3(j-1), j in [1, 683)
        k3 = {}
        k3q = [nc.sync, nc.scalar, nc.gpsimd]  # parallel issue for fast start
        with tc.tile_pool(name="ps_kp", bufs=8, space="PSUM") as ps_pool:
            ps_kp = {}
            for ic in range(8):
                for r in range(3):
                    kt = p_k3.tile([128, TC], DT, name="k3", tag="k3")
                    eng = k3q[r] if ic == 0 else nc.sync
                    eng.dma_start(kt[:], K3T[r, 128 * ic:128 * (ic + 1), :])
                    k3[(ic, r)] = kt
                    wt = p_wk.tile([128, DHALF], DT, name="wk", tag="wk")
                    nc.gpsimd.dma_start(
                        wt[:], WEK[1024 * r + 128 * ic:1024 * r + 128 * (ic + 1), :])
                    first = (ic == 0 and r == 0)
                    last = (ic == 7 and r == 2)
                    for m in range(4):
                        for ni, (t0, tw) in enumerate(KPN):
                            if first:
                                ps_kp[(m, ni)] = ps_pool.tile(
                                    [128, 341], f32, name="ps_kp", tag="ps_kp")
                            nc.tensor.matmul(
                                ps_kp[(m, ni)][:, :tw],
                                wt[:, 128 * m:128 * (m + 1)],
                                kt[:, t0:t0 + tw],
                                start=first, stop=last)
            for (m, ni), ps in ps_kp.items():
                t0, tw = KPN[ni]
                # feature rows [0:64] = head 2m, [64:128] = head 2m+1; rows
                # keep their partition placement (no shift needed)
                nc.scalar.copy(kpZ[2 * m][0:64, t0:t0 + tw], ps[0:64, :tw])
                nc.vector.tensor_copy(
                    kpZ[2 * m + 1][64:128, t0:t0 + tw], ps[64:128, :tw])

        # ---- small loads on the scalar queue, issued after the kp phase so
        # they don't compete with K3T/WEK for HBM bandwidth at startup
        mk = []
        for ti in range(8):
            mt = p_mask.tile([128, 512], DT, name="mask", tag="mask")
            nc.scalar.dma_start(mt[:], MASKB[ti])
            mk.append(mt)
        sel = []
        for t4 in range(4):
            s_t = p_small.tile([128, 128], f32r, name="sel", tag="sel", bufs=4)
            nc.scalar.dma_start(s_t[:], SEL4[t4])
            sel.append(s_t)
        wot = {}
        for kk in range(4):
            for nn2 in range(2):
                wt = p_wo.tile([128, 512], DT, name="wo", tag="wo")
                nc.scalar.dma_start(
                    wt[:], WOT[128 * kk:128 * (kk + 1), 512 * nn2:512 * (nn2 + 1)])
                wot[(kk, nn2)] = wt
        vp0 = p_small.tile([1, 520], DT, name="vp0", tag="vp0", bufs=1)
        nc.scalar.dma_start(vp0[:], VP0R[:])
        with nc.allow_non_contiguous_dma(reason="kp col-0 writes, 1KB total"):
            for h in range(H8):
                off = 64 * (h % 2)
                nc.scalar.dma_start(
                    kpZ[h][off:off + 64, 0:1], KP0[64 * h:64 * (h + 1), :])

        # ---- vp_aug fused: vpa[jb][j, 65h+c] = vp[128jb+j, 64h+c], col 65h+64=1
        with tc.tile_pool(name="ps_vp", bufs=6, space="PSUM") as ps_pool:
            ps_vp = {}
            for ic in range(8):
                for r in range(3):
                    wt = p_wv.tile([128, DHALF], DT, name="wv", tag="wv")
                    nc.gpsimd.dma_start(
                        wt[:], WEV[1024 * r + 128 * ic:1024 * r + 128 * (ic + 1), :])
                    first = (ic == 0 and r == 0)
                    last = (ic == 7 and r == 2)
                    for jb in range(NJ):
                        jr = JROWS[jb]
                        if first:
                            ps_vp[jb] = ps_pool.tile(
                                [128, 512], f32, name="ps_vp", tag="ps_vp")
                        nc.tensor.matmul(
                            ps_vp[jb][:jr, :],
                            k3[(ic, r)][:, 128 * jb:128 * jb + jr],
                            wt[:],
                            start=first, stop=last)
            for jb in range(NJ):
                jr = JROWS[jb]
                dst = vpa[jb][:jr].rearrange("p (h c) -> p h c", c=65)
                src = ps_vp[jb][:jr].rearrange("p (h c) -> p h c", c=64)
                if jb % 2 == 0:
                    nc.scalar.copy(dst[:, :, 0:64], src[:])
                else:
                    nc.vector.tensor_copy(dst[:, :, 0:64], src[:])
                nc.vector.tensor_copy(
                    dst[:, :, 64:65],
                    ones_vpa[:jr, :].rearrange("p (h c) -> p h c", c=1))
            # row 0 (key 0) comes from the host: kc[0] = k[0]
            nc.vector.tensor_copy(vpa[0][0:1, :], vp0[:])

        # ---- attention helpers (scores decoupled from o) ----
        ET = {}

        def emit_score(m, h, jc, ps_sc, s_bufs=2):
            hc = h // 2
            jr, c0 = JROWS[jc], C0[(m, jc)]
            ps = ps_sc.tile([128, 512], f32, name="ps_s", tag="ps_s",
                            bufs=s_bufs)
            nc.tensor.matmul(
                ps[:jr, c0:],
                kpZ[h][:, 128 * jc:128 * jc + jr],
                qpt[hc][:, 512 * m + c0:512 * (m + 1)],
                start=True, stop=True)
            et = p_et.tile([128, 512], DT, name="et", tag="et")
            nc.scalar.activation(et[:jr, c0:], ps[:jr, c0:], Exp, scale=SCALE)
            if (m, jc) in RAGGED:
                c1 = RAGGED[(m, jc)]
                ti = RAGGED_LIST.index((m, jc))
                nc.gpsimd.tensor_mul(
                    et[:jr, c0:c1], et[:jr, c0:c1], mk[ti][:jr, c0:c1])
            ET[(m, h, jc)] = et

        # rm zero-padded to K=128 for the full-rate bc matmul (sel rows >= 8
        # are zero, but NaN garbage in rm rows would still poison 0*NaN)
        rm = [p_rm.tile([128, 512], f32r, name="rm", tag="rm") for _ in range(4)]
        for m in range(4):
            nc.vector.memset(rm[m][:].bitcast(f32), 0.0)
        S_m = [p_rm.tile([8, 512], f32, name="sm", tag="sm") for _ in range(4)]

        def emit_o_mm(m, h, jc, po, first, last):
            jr, c0 = JROWS[jc], C0[(m, jc)]
            nc.tensor.matmul(
                po[:65, c0:],
                vpa[jc][:jr, 65 * h:65 * (h + 1)],
                ET[(m, h, jc)][:jr, c0:],
                start=first, stop=last)

        def emit_o_post(m, h, po):
            hc, off = h // 2, (h % 2) * 64
            ss = p_ss.tile([1, 512], f32, name="ss", tag="ss", bufs=2)
            nc.vector.tensor_copy(ss[:], po[64:65, :])
            nc.sync.dma_start(S_m[m][h:h + 1, :], ss[:])
            nc.vector.tensor_copy(o_nt[(hc, m)][off:off + 64, :], po[0:64, :])
            if h == H8 - 1:
                rt = p_ss.tile([8, 512], f32, name="rt", tag="rt", bufs=1)
                nc.vector.reciprocal_approx_fast(out=rt[:], in_=S_m[m][:])
                nc.vector.tensor_copy(rm[m][0:8, :], rt[:])

        def emit_o(m, h, ps_att, o_bufs=2):
            js = JCS[m]
            po = ps_att.tile([128, 512], f32, name="ps_o", tag="ps_o",
                             bufs=o_bufs)
            for jc in js:
                emit_o_mm(m, h, jc, po, jc == js[0], jc == js[-1])
            emit_o_post(m, h, po)

        def emit_bc(m, ps_att):
            for t4 in range(4):
                bc = ps_att.tile([128, 512], f32, name="ps_bc", tag="ps_bc",
                                 bufs=2)
                nc.tensor.matmul(bc[:], sel[t4][:], rm[m][:], start=True,
                                 stop=True)
                for half in range(2):
                    dst = o_nt[(t4, m)][64 * half:64 * half + 64, :]
                    nc.vector.tensor_mul(
                        dst, dst, bc[64 * half:64 * half + 64, :])

        def emit_fin_mq(m, mq, ps_att):
            ob = p_out.tile([128, D], f32, name="outsb", tag="outsb")
            rows = slice(512 * m + 128 * mq, 512 * m + 128 * (mq + 1))
            for nn2 in range(2):
                pf = ps_att.tile([128, 512], f32, name="ps_f", tag="ps_f",
                                 bufs=2)
                for kk in range(4):
                    nc.tensor.matmul(
                        pf[:],
                        o_nt[(kk, m)][:, 128 * mq:128 * (mq + 1)],
                        wot[(kk, nn2)][:],
                        start=(kk == 0), stop=(kk == 3))
                # scalar still runs the exp stream while m=0/1 finalize
                if m < 2:
                    nc.vector.tensor_copy(ob[:, 512 * nn2:512 * (nn2 + 1)], pf[:])
                else:
                    nc.scalar.copy(ob[:, 512 * nn2:512 * (nn2 + 1)], pf[:])
            dq = (nc.gpsimd, nc.sync)[mq % 2]
            dq.dma_start(OUT[rows, :], ob[:])

        # ---- qp passes; scores(m=pass-1) and o(pass-2) interleaved ----
        # PSUM per pass: ps_qp 4 + ps_s 2 + ps_o 2 = 8 banks.
        with tc.tile_pool(name="ps_qp", bufs=8, space="PSUM") as ps_pool:
            for npass in range(4):
                sc_chunks = [[] for _ in range(8)]
                if npass >= 1:
                    msc = npass - 1
                    tiles = [(h, jc) for h in range(H8) for jc in JCS[msc]]
                    sc_chunks = chunk_even(tiles, 8)
                ps_qp = {}
                for kk in range(8):
                    wq_t = p_wq.tile([128, DHALF], DT, name="wq", tag="wq")
                    nc.sync.dma_start(wq_t[:], WQT[128 * kk:128 * (kk + 1), :])
                    qt_t = p_qt.tile([128, 512], DT, name="qt", tag="qt")
                    nc.sync.dma_start(
                        qt_t[:],
                        qT[128 * kk:128 * (kk + 1), 512 * npass:512 * (npass + 1)])
                    # zip qp matmuls with score tiles so the exp stream always
                    # has non-dependent PE work between score matmuls
                    sc_it = list(sc_chunks[kk])
                    for m in range(4):
                        if kk == 0:
                            ps_qp[m] = ps_pool.tile(
                                [128, 512], f32, name="ps_qp", tag="ps_qp",
                                bufs=4)
                        nc.tensor.matmul(
                            ps_qp[m][:],
                            wq_t[:, 128 * m:128 * (m + 1)],
                            qt_t[:],
                            start=(kk == 0), stop=(kk == 7))
                        if sc_it:
                            h, jc = sc_it.pop(0)
                            emit_score(npass - 1, h, jc, ps_pool)
                    for (h, jc) in sc_it:
                        emit_score(npass - 1, h, jc, ps_pool)
                    if npass >= 2:
                        emit_o(npass - 2, kk, ps_pool)
                for m, ps in ps_qp.items():
                    if m % 2 == 0:
                        nc.scalar.copy(
                            qpt[m][:, 512 * npass:512 * (npass + 1)], ps[:])
                    else:
                        nc.vector.tensor_copy(
                            qpt[m][:, 512 * npass:512 * (npass + 1)], ps[:])

        # ---- tail: sc(3) / o(2) / o(3) / finalize pipelined per head ----
        # PSUM: ps_s 2 + ps_o 2 + ps_bc 2 + ps_f 2 = 8 banks.
        with tc.tile_pool(name="ps_att", bufs=2, space="PSUM") as ps_att:
            fin_chunks = []  # deferred finalize chunks for m = 0, 1
            fin_chunks.append(lambda: emit_bc(0, ps_att))
            for mq in range(4):
                fin_chunks.append(lambda m=0, q=mq: emit_fin_mq(m, q, ps_att))
            fin_chunks.append(lambda: emit_bc(1, ps_att))
            for mq in range(4):
                fin_chunks.append(lambda m=1, q=mq: emit_fin_mq(m, q, ps_att))
            ci = 0
            for h in range(H8):
                # zip sc3 / o2 / o3 matmuls so exp latency hides behind
                # independent o-matmuls instead of gating ps_s recycling
                js3, js2 = JCS[3], JCS[2]
                po2 = ps_att.tile([128, 512], f32, name="ps_o", tag="ps_o",
                                  bufs=2)
                emit_score(3, h, js3[0], ps_att)
                emit_score(3, h, js3[1], ps_att)
                for i, jc in enumerate(js2):
                    emit_o_mm(2, h, jc, po2, i == 0, i == len(js2) - 1)
                emit_score(3, h, js3[2], ps_att)
                emit_score(3, h, js3[3], ps_att)
                if h >= 1:
                    po3 = ps_att.tile([128, 512], f32, name="ps_o",
                                      tag="ps_o", bufs=2)
                    for i, jc in enumerate(js3):
                        emit_o_mm(3, h - 1, jc, po3, i == 0, i == 5)
                emit_score(3, h, js3[4], ps_att)
                emit_score(3, h, js3[5], ps_att)
                emit_o_post(2, h, po2)
                if h >= 1:
                    emit_o_post(3, h - 1, po3)
                # 1-2 finalize chunks per slot; bc(1) needs o(1) (done in qp
                # pass 3), all fit behind the sc3/o2/o3 stream
                for _ in range(2 if h >= 4 else 1):
                    if ci < len(fin_chunks):
                        fin_chunks[ci]()
                        ci += 1
            while ci < len(fin_chunks):
                fin_chunks[ci]()
                ci += 1
            emit_bc(2, ps_att)
            emit_o(3, 7, ps_att)
            emit_fin_mq(2, 0, ps_att)
            emit_fin_mq(2, 1, ps_att)
            # fin(3): all 4 bc tiles live at once (borrow the now-idle ps_s /
            # ps_o rings), normalize per 128-col slice, and interleave with
            # the remaining fin(2) chunks so mq3 starts as early as possible
            bc3 = []
            for t4, tag in enumerate(("ps_bc", "ps_bc", "ps_s", "ps_o")):
                bc = ps_att.tile([128, 512], f32, name="ps_bc3", tag=tag,
                                 bufs=2)
                nc.tensor.matmul(bc[:], sel[t4][:], rm[3][:], start=True,
                                 stop=True)
                bc3.append(bc)
            emit_fin_mq(2, 2, ps_att)

            def norm3(mq):
                for t4 in range(4):
                    for half in range(2):
                        dst = o_nt[(t4, 3)][64 * half:64 * half + 64,
                                            128 * mq:128 * (mq + 1)]
                        nc.vector.tensor_mul(
                            dst, dst,
                            bc3[t4][64 * half:64 * half + 64,
                                    128 * mq:128 * (mq + 1)])

            norm3(0)
            emit_fin_mq(2, 3, ps_att)
            norm3(1)
            emit_fin_mq(3, 0, ps_att)
            norm3(2)
            emit_fin_mq(3, 1, ps_att)
            norm3(3)
            emit_fin_mq(3, 2, ps_att)
            emit_fin_mq(3, 3, ps_att)

    return nc


def make_maskb():
    import ml_dtypes
    mask = np.zeros((8, 128, 512), dtype=np.float32)
    for t, (m, jc) in enumerate(RAGGED_LIST):
        qq = 512 * m + np.arange(512)[None, :]
        jj = 128 * jc + np.arange(128)[:, None]
        mask[t] = (3 * jj <= qq).astype(np.float32)  # 1.0 where visible
    return mask.astype(ml_dtypes.bfloat16)


def make_sel4():
    sel = np.zeros((4, 128, 128), dtype=np.float32)
    for t in range(4):
        for r in range(128):
            sel[t, 2 * t + r // 64, r] = 1.0
    return sel


def make_k3t(kb):
    """kb: [T, D] f32 for one batch -> K3T [3, D, TC] bf16.
    K3T[r, i, j] = k[3(j-1)+r, i] for j >= 1; column 0 is zero."""
    import ml_dtypes
    out = np.zeros((3, D, TC), dtype=np.float32)
    for r in range(3):
        out[r, :, 1:] = kb[r:r + 3 * (TC - 1):3, :].T
    return out.astype(ml_dtypes.bfloat16)


def prep_inputs(q, k, Wq, Wk, Wv, Wo, conv_w):
    """Returns list of 8 in_maps (core c = 2b + g)."""
    import ml_dtypes
    bf = ml_dtypes.bfloat16
    # Wc3T[r*1024 + i, o] = conv_w[o, i, r]  (so kc[j] = Wc3T.T @ k3(j-1))
    Wc3T = np.ascontiguousarray(
        conv_w.transpose(2, 1, 0).reshape(3 * D, D)).astype(np.float32)
    maskb = make_maskb()
    sel4 = make_sel4()
    halves = []
    for g in range(2):
        sl = slice(DHALF * g, DHALF * (g + 1))
        WEKg = np.ascontiguousarray((Wc3T @ Wk[sl].T)).astype(bf)
        WEVg = np.ascontiguousarray((Wc3T @ Wv[sl].T)).astype(bf)
        halves.append((sl, WEKg, WEVg))
    k3ts = [make_k3t(k[b]) for b in range(B)]
    in_maps = []
    for c in range(8):
        b, g = c // 2, c % 2
        sl, WEKg, WEVg = halves[g]
        kp0 = (Wk[sl] @ k[b, 0]).astype(np.float32).reshape(DHALF, 1)
        vp0 = (Wv[sl] @ k[b, 0]).astype(np.float32)
        vp0r = np.zeros((8, 65), np.float32)
        vp0r[:, :64] = vp0.reshape(8, 64)
        vp0r[:, 64] = 1.0
        in_maps.append({
            "qT": np.ascontiguousarray(q[b].T).astype(bf),
            "K3T": k3ts[b],
            "WEK": WEKg,
            "WEV": WEVg,
            "WQT": np.ascontiguousarray(Wq[sl, :].T).astype(bf),
            "WOT": np.ascontiguousarray(Wo[:, sl].T).astype(bf),
            "MASKB": maskb,
            "SEL4": sel4,
            "KP0": kp0.astype(bf),
            "VP0R": vp0r.reshape(1, 520).astype(bf),
        })
    return in_maps


def postprocess(results, bo):
    out = np.zeros((B, T, D), dtype=np.float32)
    for b in range(B):
        out[b] = (np.asarray(results[2 * b]["out_p"], dtype=np.float32)
                  + np.asarray(results[2 * b + 1]["out_p"], dtype=np.float32)
                  + bo[None, :])
    return out


_CACHED_NC = None


def kernel(q, k, v, Wq, Wk, Wv, Wo, bo, conv_w):
    """Full-input entry point. v is unused by the reference computation
    (V is replaced by the conv-compressed K)."""
    global _CACHED_NC
    from concourse.bass_utils import run_bass_kernel_spmd

    q = np.asarray(q, dtype=np.float32)
    k = np.asarray(k, dtype=np.float32)
    Wq = np.asarray(Wq, dtype=np.float32)
    Wk = np.asarray(Wk, dtype=np.float32)
    Wv = np.asarray(Wv, dtype=np.float32)
    Wo = np.asarray(Wo, dtype=np.float32)
    bo = np.asarray(bo, dtype=np.float32)
    conv_w = np.asarray(conv_w, dtype=np.float32)

    in_maps = prep_inputs(q, k, Wq, Wk, Wv, Wo, conv_w)
    if _CACHED_NC is None:
        nc = build_nc()
        nc.finalize()
        _CACHED_NC = nc
    res = run_bass_kernel_spmd(_CACHED_NC, in_maps, list(range(8)))
    return postprocess(res.results, bo)


# revision 67
# speedup vs baseline: 1.0430x; 1.0012x over previous
"""Sparse (conv-compressed) multi-head attention on 8 Trainium2 NeuronCores.

Entry point: kernel(**inputs) -> np.ndarray [4, 2048, 1024] float32.

Sharding: core c = 2*b + g  (b = batch 0..3, g = head-half 0..1).
Each core: batch b, heads [8g, 8g+8), all 2048 queries.
Final projection produces a partial (dv-half contraction); host sums pairs + bias.

v3 design notes:
- The strided conv that builds the compressed keys kc is FUSED into the
  Wk/Wv projections on the host:  kp = (Wk_hh @ Wc3) @ k3,  vp likewise,
  where k3(t) = concat(k[3t], k[3t+1], k[3t+2]).  No conv intermediate,
  no cross-core collective.  Column j=0 (kc[0] = k[0]) comes from the host.
- The host ships kT pre-decimated as K3T[r, i, j] = k[3(j-1)+r, i] so every
  matmul operand is contiguous (strided SBUF reads cost ~1.5x on the PE).
- Score matmuls + exp are decoupled from the attention-value matmuls: scores
  stream into the qp projection passes so the scalar engine's exp pipeline
  (the second-longest serial resource) starts ~60us earlier and never paces
  the PE.  o/normalize/output-projection work is interleaved per q-tile at
  the tail.
- Causal mask applied as a 0/1 bf16 multiply on eT after exp (gpsimd).
"""
import sys
sys.path.insert(0, '/opt/trn_rl_repo')
import numpy as np
import concourse.bass as bass
import concourse.bacc as bacc
import concourse.mybir as mybir
from concourse import tile
from contextlib import ExitStack

f32 = mybir.dt.float32
f32r = mybir.dt.float32r
bf16 = mybir.dt.bfloat16
DT = bf16
Exp = mybir.ActivationFunctionType.Exp

B, T, D, H = 4, 2048, 1024, 16
DH = 64
TC = 683          # compressed keys: 1 + 682
DHALF = D // 2    # per-core head-half width
H8 = H // 2       # heads per core
SCALE = DH ** -0.5  # 0.125

# kp n-splits over keys [1, 683)
KPN = [(1, 341), (342, 341)]

# attention q-tiles (4 x 512) and j-chunks (6 x 128, last = 43 rows)
NJ = 6
JROWS = [128, 128, 128, 128, 128, TC - 5 * 128]  # last = 43
JCS = {m: [jc for jc in range(NJ) if 384 * jc < 512 * (m + 1)] for m in range(4)}
C0 = {(m, jc): max(0, 384 * jc - 512 * m) for m in range(4) for jc in JCS[m]}
# ragged tiles (m, jc) -> first fully-visible column c1 (cols [c0, c1) get mask)
RAGGED = {}
for m in range(4):
    for jc in JCS[m]:
        if not (384 * jc + 381 <= 512 * m):
            RAGGED[(m, jc)] = min(512, 384 * jc + 381 - 512 * m)
RAGGED_LIST = sorted(RAGGED.keys())  # 8 tiles
assert len(RAGGED_LIST) == 8


def chunk_even(lst, n):
    """Split lst into n chunks with sizes as even as possible."""
    k, r = divmod(len(lst), n)
    out, i = [], 0
    for j in range(n):
        sz = k + (1 if j < r else 0)
        out.append(lst[i:i + sz])
        i += sz
    return out


def build_nc():
    nc = bacc.Bacc(None, target_bir_lowering=False, debug=False)

    qT = nc.dram_tensor("qT", [D, T], DT, kind="ExternalInput")
    K3T = nc.dram_tensor("K3T", [3, D, TC], DT, kind="ExternalInput")
    WEK = nc.dram_tensor("WEK", [3 * D, DHALF], DT, kind="ExternalInput")
    WEV = nc.dram_tensor("WEV", [3 * D, DHALF], DT, kind="ExternalInput")
    WQT = nc.dram_tensor("WQT", [D, DHALF], DT, kind="ExternalInput")
    WOT = nc.dram_tensor("WOT", [DHALF, D], DT, kind="ExternalInput")
    MASKB = nc.dram_tensor("MASKB", [8, 128, 512], DT, kind="ExternalInput")
    SEL4 = nc.dram_tensor("SEL4", [4, 128, 128], f32r, kind="ExternalInput")
    KP0 = nc.dram_tensor("KP0", [DHALF, 1], DT, kind="ExternalInput")
    VP0R = nc.dram_tensor("VP0R", [1, 520], DT, kind="ExternalInput")
    OUT = nc.dram_tensor("out_p", [T, D], f32, kind="ExternalOutput")

    with tile.TileContext(nc) as tc, ExitStack() as st:
        st.enter_context(nc.allow_low_precision("bf16 matmuls, f32r recip bcast"))
        pool = lambda **kw: st.enter_context(tc.tile_pool(**kw))
        p_k3 = pool(name="k3", bufs=24)       # K3T chunks [128, 683]
        p_wk = pool(name="wk", bufs=6)        # WEK chunks [128, 512]
        p_wv = pool(name="wv", bufs=6)        # WEV chunks [128, 512]
        p_wq = pool(name="wq", bufs=8)        # WQT chunks [128, 512]
        p_qt = pool(name="qt", bufs=8)        # qT chunks [128, 512]
        p_kpt = pool(name="kpt", bufs=8)      # kp^T per head, K zero-padded
        p_vpa = pool(name="vpa", bufs=6)      # vp_aug [128, 520]
        p_qpt = pool(name="qpt", bufs=4)      # qp^T [128, 2048]
        p_mask = pool(name="mask", bufs=8)    # 0/1 bf16 masks [128, 512]
        p_et = pool(name="et", bufs=42)       # exp(scores) [128, 512] bf16
        p_ont = pool(name="ont", bufs=16)     # unnormalized head outs [128, 512]
        p_wo = pool(name="wo", bufs=8)        # WoT resident [128, 512]
        p_out = pool(name="outsb", bufs=3)    # out staging [128, 1024] f32
        p_rm = pool(name="rm", bufs=4)        # per-m recip denominators [8, 512]
        p_ss = pool(name="ss", bufs=8)        # denom staging rows [1, 512]
        p_small = pool(name="small", bufs=6)

        # kpZ[h]: rows 0-63 = head h's kp dims, rows 64-127 = ZERO.  Scores
        # then contract K=128 (full-rate: K=64 matmuls stream at half speed);
        # the zero rows null out the other head's qp rows in the rhs.
        # head h lives in qpt partition rows [64*(h%2), +64); kpZ mirrors that
        # row placement and zeroes the other head's rows
        kpZ = [p_kpt.tile([128, TC], DT, name="kpt", tag="kpt") for _ in range(H8)]
        for h in range(H8):
            z0 = 64 * ((h + 1) % 2)
            nc.vector.memset(kpZ[h][z0:z0 + 64, :], 0.0)
        vpa = [p_vpa.tile([128, 520], DT, name="vpa", tag="vpa") for _ in range(NJ)]
        qpt = [p_qpt.tile([128, T], DT, name="qpt", tag="qpt") for _ in range(4)]
        o_nt = {(kk, m): p_ont.tile([128, 512], DT, name="ont", tag="ont")
                for kk in range(4) for m in range(4)}
        ones_vpa = p_small.tile([128, 8], f32, name="ones_vpa", tag="ones_vpa",
                                bufs=1)
        nc.vector.memset(ones_vpa[:], 1.0)

        # ---- kp^T fused: kpt[m][:, j] = (Wk_hh @ Wc3) @ k3(j-1), j in [1, 683)
        k3 = {}
        k3q = [nc.sync, nc.scalar, nc.gpsimd]  # parallel issue for fast start
        with tc.tile_pool(name="ps_kp", bufs=8, space="PSUM") as ps_pool:
            ps_kp = {}
            for ic in range(8):
                for r in range(3):
                    kt = p_k3.tile([128, TC], DT, name="k3", tag="k3")
                    wt = p_wk.tile([128, DHALF], DT, name="wk", tag="wk")
                    # wek first on gpsimd: k3(0,2) isn't needed until the
                    # third chunk block, wek(0,0) gates the first matmul
                    nc.gpsimd.dma_start(
                        wt[:], WEK[1024 * r + 128 * ic:1024 * r + 128 * (ic + 1), :])
                    eng = k3q[r] if ic == 0 else nc.sync
                    eng.dma_start(kt[:], K3T[r, 128 * ic:128 * (ic + 1), :])
                    k3[(ic, r)] = kt
                    first = (ic == 0 and r == 0)
                    last = (ic == 7 and r == 2)
                    for m in range(4):
                        for ni, (t0, tw) in enumerate(KPN):
                            if first:
                                ps_kp[(m, ni)] = ps_pool.tile(
                                    [128, 341], f32, name="ps_kp", tag="ps_kp")
                            nc.tensor.matmul(
                                ps_kp[(m, ni)][:, :tw],
                                wt[:, 128 * m:128 * (m + 1)],
                                kt[:, t0:t0 + tw],
                                start=first, stop=last)
            for (m, ni), ps in ps_kp.items():
                t0, tw = KPN[ni]
                # feature rows [0:64] = head 2m, [64:128] = head 2m+1; rows
                # keep their partition placement (no shift needed)
                nc.scalar.copy(kpZ[2 * m][0:64, t0:t0 + tw], ps[0:64, :tw])
                nc.vector.tensor_copy(
                    kpZ[2 * m + 1][64:128, t0:t0 + tw], ps[64:128, :tw])

        # ---- small loads on the scalar queue, issued after the kp phase so
        # they don't compete with K3T/WEK for HBM bandwidth at startup
        mk = []
        for ti in range(8):
            mt = p_mask.tile([128, 512], DT, name="mask", tag="mask")
            nc.scalar.dma_start(mt[:], MASKB[ti])
            mk.append(mt)
        sel = []
        for t4 in range(4):
            s_t = p_small.tile([128, 128], f32r, name="sel", tag="sel", bufs=4)
            nc.scalar.dma_start(s_t[:], SEL4[t4])
            sel.append(s_t)
        wot = {}
        for kk in range(4):
            for nn2 in range(2):
                wt = p_wo.tile([128, 512], DT, name="wo", tag="wo")
                nc.scalar.dma_start(
                    wt[:], WOT[128 * kk:128 * (kk + 1), 512 * nn2:512 * (nn2 + 1)])
                wot[(kk, nn2)] = wt
        vp0 = p_small.tile([1, 520], DT, name="vp0", tag="vp0", bufs=1)
        nc.scalar.dma_start(vp0[:], VP0R[:])
        with nc.allow_non_contiguous_dma(reason="kp col-0 writes, 1KB total"):
            for h in range(H8):
                off = 64 * (h % 2)
                nc.scalar.dma_start(
                    kpZ[h][off:off + 64, 0:1], KP0[64 * h:64 * (h + 1), :])

        # ---- vp_aug fused: vpa[jb][j, 65h+c] = vp[128jb+j, 64h+c], col 65h+64=1
        with tc.tile_pool(name="ps_vp", bufs=6, space="PSUM") as ps_pool:
            ps_vp = {}
            for ic in range(8):
                for r in range(3):
                    wt = p_wv.tile([128, DHALF], DT, name="wv", tag="wv")
                    nc.gpsimd.dma_start(
                        wt[:], WEV[1024 * r + 128 * ic:1024 * r + 128 * (ic + 1), :])
                    first = (ic == 0 and r == 0)
                    last = (ic == 7 and r == 2)
                    for jb in range(NJ):
                        jr = JROWS[jb]
                        if first:
                            ps_vp[jb] = ps_pool.tile(
                                [128, 512], f32, name="ps_vp", tag="ps_vp")
                        nc.tensor.matmul(
                            ps_vp[jb][:jr, :],
                            k3[(ic, r)][:, 128 * jb:128 * jb + jr],
                            wt[:],
                            start=first, stop=last)
            for jb in range(NJ):
                jr = JROWS[jb]
                dst = vpa[jb][:jr].rearrange("p (h c) -> p h c", c=65)
                src = ps_vp[jb][:jr].rearrange("p (h c) -> p h c", c=64)
                if jb % 2 == 0:
                    nc.scalar.copy(dst[:, :, 0:64], src[:])
                else:
                    nc.vector.tensor_copy(dst[:, :, 0:64], src[:])
                nc.vector.tensor_copy(
                    dst[:, :, 64:65],
                    ones_vpa[:jr, :].rearrange("p (h c) -> p h c", c=1))
            # row 0 (key 0) comes from the host: kc[0] = k[0]
            nc.vector.tensor_copy(vpa[0][0:1, :], vp0[:])

        # ---- attention helpers (scores decoupled from o) ----
        ET = {}

        def emit_score(m, h, jc, ps_sc, s_bufs=2):
            hc = h // 2
            jr, c0 = JROWS[jc], C0[(m, jc)]
            ps = ps_sc.tile([128, 512], f32, name="ps_s", tag="ps_s",
                            bufs=s_bufs)
            nc.tensor.matmul(
                ps[:jr, c0:],
                kpZ[h][:, 128 * jc:128 * jc + jr],
                qpt[hc][:, 512 * m + c0:512 * (m + 1)],
                start=True, stop=True)
            et = p_et.tile([128, 512], DT, name="et", tag="et")
            nc.scalar.activation(et[:jr, c0:], ps[:jr, c0:], Exp, scale=SCALE)
            if (m, jc) in RAGGED:
                c1 = RAGGED[(m, jc)]
                ti = RAGGED_LIST.index((m, jc))
                nc.gpsimd.tensor_mul(
                    et[:jr, c0:c1], et[:jr, c0:c1], mk[ti][:jr, c0:c1])
            ET[(m, h, jc)] = et

        # rm zero-padded to K=128 for the full-rate bc matmul (sel rows >= 8
        # are zero, but NaN garbage in rm rows would still poison 0*NaN)
        rm = [p_rm.tile([128, 512], f32r, name="rm", tag="rm") for _ in range(4)]
        for m in range(4):
            nc.vector.memset(rm[m][:].bitcast(f32), 0.0)
        S_m = [p_rm.tile([8, 512], f32, name="sm", tag="sm") for _ in range(4)]

        def emit_o_mm(m, h, jc, po, first, last):
            jr, c0 = JROWS[jc], C0[(m, jc)]
            nc.tensor.matmul(
                po[:65, c0:],
                vpa[jc][:jr, 65 * h:65 * (h + 1)],
                ET[(m, h, jc)][:jr, c0:],
                start=first, stop=last)

        def emit_o_post(m, h, po):
            hc, off = h // 2, (h % 2) * 64
            ss = p_ss.tile([1, 512], f32, name="ss", tag="ss", bufs=2)
            nc.vector.tensor_copy(ss[:], po[64:65, :])
            nc.sync.dma_start(S_m[m][h:h + 1, :], ss[:])
            if h == H8 - 1:
                # recip gates bc(m); run it ahead of the (non-urgent) o_nt copy
                rt = p_ss.tile([8, 512], f32, name="rt", tag="rt", bufs=1)
                nc.vector.reciprocal_approx_fast(out=rt[:], in_=S_m[m][:])
                nc.vector.tensor_copy(rm[m][0:8, :], rt[:])
            nc.vector.tensor_copy(o_nt[(hc, m)][off:off + 64, :], po[0:64, :])

        def emit_o(m, h, ps_att, o_bufs=2):
            js = JCS[m]
            po = ps_att.tile([128, 512], f32, name="ps_o", tag="ps_o",
                             bufs=o_bufs)
            for jc in js:
                emit_o_mm(m, h, jc, po, jc == js[0], jc == js[-1])
            emit_o_post(m, h, po)

        def emit_bc(m, ps_att):
            for t4 in range(4):
                bc = ps_att.tile([128, 512], f32, name="ps_bc", tag="ps_bc",
                                 bufs=2)
                nc.tensor.matmul(bc[:], sel[t4][:], rm[m][:], start=True,
                                 stop=True)
                for half in range(2):
                    dst = o_nt[(t4, m)][64 * half:64 * half + 64, :]
                    nc.vector.tensor_mul(
                        dst, dst, bc[64 * half:64 * half + 64, :])

        def emit_fin_mq(m, mq, ps_att):
            ob = p_out.tile([128, D], f32, name="outsb", tag="outsb")
            rows = slice(512 * m + 128 * mq, 512 * m + 128 * (mq + 1))
            for nn2 in range(2):
                pf = ps_att.tile([128, 512], f32, name="ps_f", tag="ps_f",
                                 bufs=2)
                for kk in range(4):
                    nc.tensor.matmul(
                        pf[:],
                        o_nt[(kk, m)][:, 128 * mq:128 * (mq + 1)],
                        wot[(kk, nn2)][:],
                        start=(kk == 0), stop=(kk == 3))
                # scalar still runs the exp stream while m=0/1 finalize
                if m < 2:
                    nc.vector.tensor_copy(ob[:, 512 * nn2:512 * (nn2 + 1)], pf[:])
                else:
                    nc.scalar.copy(ob[:, 512 * nn2:512 * (nn2 + 1)], pf[:])
            dq = (nc.gpsimd, nc.sync)[mq % 2]
            dq.dma_start(OUT[rows, :], ob[:])

        # ---- qp passes; scores(m=pass-1) and o(pass-2) interleaved ----
        # PSUM per pass: ps_qp 4 + ps_s 2 + ps_o 2 = 8 banks.
        with tc.tile_pool(name="ps_qp", bufs=8, space="PSUM") as ps_pool:
            for npass in range(4):
                sc_chunks = [[] for _ in range(8)]
                if npass >= 1:
                    msc = npass - 1
                    tiles = [(h, jc) for h in range(H8) for jc in JCS[msc]]
                    sc_chunks = chunk_even(tiles, 8)
                ps_qp = {}
                for kk in range(8):
                    wq_t = p_wq.tile([128, DHALF], DT, name="wq", tag="wq")
                    nc.sync.dma_start(wq_t[:], WQT[128 * kk:128 * (kk + 1), :])
                    qt_t = p_qt.tile([128, 512], DT, name="qt", tag="qt")
                    nc.sync.dma_start(
                        qt_t[:],
                        qT[128 * kk:128 * (kk + 1), 512 * npass:512 * (npass + 1)])
                    # zip qp matmuls with score tiles so the exp stream always
                    # has non-dependent PE work between score matmuls
                    sc_it = list(sc_chunks[kk])
                    for m in range(4):
                        if kk == 0:
                            ps_qp[m] = ps_pool.tile(
                                [128, 512], f32, name="ps_qp", tag="ps_qp",
                                bufs=4)
                        nc.tensor.matmul(
                            ps_qp[m][:],
                            wq_t[:, 128 * m:128 * (m + 1)],
                            qt_t[:],
                            start=(kk == 0), stop=(kk == 7))
                        if sc_it:
                            h, jc = sc_it.pop(0)
                            emit_score(npass - 1, h, jc, ps_pool)
                    for (h, jc) in sc_it:
                        emit_score(npass - 1, h, jc, ps_pool)
                    if npass >= 2:
                        emit_o(npass - 2, kk, ps_pool)
                for m, ps in ps_qp.items():
                    if m % 2 == 0:
                        nc.scalar.copy(
                            qpt[m][:, 512 * npass:512 * (npass + 1)], ps[:])
                    else:
                        nc.vector.tensor_copy(
                            qpt[m][:, 512 * npass:512 * (npass + 1)], ps[:])

        # ---- tail: sc(3) / o(2) / o(3) / finalize pipelined per head ----
        # PSUM: ps_s 2 + ps_o 2 + ps_bc 2 + ps_f 2 = 8 banks.
        with tc.tile_pool(name="ps_att", bufs=2, space="PSUM") as ps_att:
            fin_chunks = []  # deferred finalize chunks for m = 0, 1
            fin_chunks.append(lambda: emit_bc(0, ps_att))
            for mq in range(4):
                fin_chunks.append(lambda m=0, q=mq: emit_fin_mq(m, q, ps_att))
            fin_chunks.append(lambda: emit_bc(1, ps_att))
            for mq in range(4):
                fin_chunks.append(lambda m=1, q=mq: emit_fin_mq(m, q, ps_att))
            ci = 0
            for h in range(H8):
                # zip sc3 / o2 / o3 matmuls so exp latency hides behind
                # independent o-matmuls instead of gating ps_s recycling
                js3, js2 = JCS[3], JCS[2]
                po2 = ps_att.tile([128, 512], f32, name="ps_o", tag="ps_o",
                                  bufs=2)
                emit_score(3, h, js3[0], ps_att)
                emit_score(3, h, js3[1], ps_att)
                for i, jc in enumerate(js2):
                    emit_o_mm(2, h, jc, po2, i == 0, i == len(js2) - 1)
                emit_score(3, h, js3[2], ps_att)
                emit_score(3, h, js3[3], ps_att)
                if h >= 1:
                    po3 = ps_att.tile([128, 512], f32, name="ps_o",
                                      tag="ps_o", bufs=2)
                    for i, jc in enumerate(js3):
                        emit_o_mm(3, h - 1, jc, po3, i == 0, i == 5)
                emit_score(3, h, js3[4], ps_att)
                emit_score(3, h, js3[5], ps_att)
                emit_o_post(2, h, po2)
                if h >= 1:
                    emit_o_post(3, h - 1, po3)
                # 1-2 finalize chunks per slot; bc(1) needs o(1) (done in qp
                # pass 3), all fit behind the sc3/o2/o3 stream
                for _ in range(2 if h >= 4 else 1):
                    if ci < len(fin_chunks):
                        fin_chunks[ci]()
                        ci += 1
            while ci < len(fin_chunks):
                fin_chunks[ci]()
                ci += 1
            emit_o(3, 7, ps_att)
            emit_bc(2, ps_att)
            emit_fin_mq(2, 0, ps_att)
            emit_fin_mq(2, 1, ps_att)
            # fin(3): all 4 bc tiles live at once (borrow the now-idle ps_s /
            # ps_o rings), normalize per 128-col slice, and interleave with
            # the remaining fin(2) chunks so mq3 starts as early as possible
            bc3 = []
            for t4, tag in enumerate(("ps_bc", "ps_bc", "ps_s", "ps_o")):
                bc = ps_att.tile([128, 512], f32, name="ps_bc3", tag=tag,
                                 bufs=2)
                nc.tensor.matmul(bc[:], sel[t4][:], rm[3][:], start=True,
                                 stop=True)
                bc3.append(bc)
            emit_fin_mq(2, 2, ps_att)

            def norm3(mq):
                for t4 in range(4):
                    for half in range(2):
                        dst = o_nt[(t4, 3)][64 * half:64 * half + 64,
                                            128 * mq:128 * (mq + 1)]
                        nc.vector.tensor_mul(
                            dst, dst,
                            bc3[t4][64 * half:64 * half + 64,
                                    128 * mq:128 * (mq + 1)])

            norm3(0)
            emit_fin_mq(2, 3, ps_att)
            norm3(1)
            emit_fin_mq(3, 0, ps_att)
            norm3(2)
            emit_fin_mq(3, 1, ps_att)
            norm3(3)
            emit_fin_mq(3, 2, ps_att)
            emit_fin_mq(3, 3, ps_att)

    return nc


def make_maskb():
    import ml_dtypes
    mask = np.zeros((8, 128, 512), dtype=np.float32)
    for t, (m, jc) in enumerate(RAGGED_LIST):
        qq = 512 * m + np.arange(512)[None, :]
        jj = 128 * jc + np.arange(128)[:, None]
        mask[t] = (3 * jj <= qq).astype(np.float32)  # 1.0 where visible
    return mask.astype(ml_dtypes.bfloat16)


def make_sel4():
    sel = np.zeros((4, 128, 128), dtype=np.float32)
    for t in range(4):
        for r in range(128):
            sel[t, 2 * t + r // 64, r] = 1.0
    return sel


def make_k3t(kb):
    """kb: [T, D] f32 for one batch -> K3T [3, D, TC] bf16.
    K3T[r, i, j] = k[3(j-1)+r, i] for j >= 1; column 0 is zero."""
    import ml_dtypes
    out = np.zeros((3, D, TC), dtype=np.float32)
    for r in range(3):
        out[r, :, 1:] = kb[r:r + 3 * (TC - 1):3, :].T
    return out.astype(ml_dtypes.bfloat16)


def prep_inputs(q, k, Wq, Wk, Wv, Wo, conv_w):
    """Returns list of 8 in_maps (core c = 2b + g)."""
    import ml_dtypes
    bf = ml_dtypes.bfloat16
    # Wc3T[r*1024 + i, o] = conv_w[o, i, r]  (so kc[j] = Wc3T.T @ k3(j-1))
    Wc3T = np.ascontiguousarray(
        conv_w.transpose(2, 1, 0).reshape(3 * D, D)).astype(np.float32)
    maskb = make_maskb()
    sel4 = make_sel4()
    halves = []
    for g in range(2):
        sl = slice(DHALF * g, DHALF * (g + 1))
        WEKg = np.ascontiguousarray((Wc3T @ Wk[sl].T)).astype(bf)
        WEVg = np.ascontiguousarray((Wc3T @ Wv[sl].T)).astype(bf)
        halves.append((sl, WEKg, WEVg))
    k3ts = [make_k3t(k[b]) for b in range(B)]
    in_maps = []
    for c in range(8):
        b, g = c // 2, c % 2
        sl, WEKg, WEVg = halves[g]
        kp0 = (Wk[sl] @ k[b, 0]).astype(np.float32).reshape(DHALF, 1)
        vp0 = (Wv[sl] @ k[b, 0]).astype(np.float32)
        vp0r = np.zeros((8, 65), np.float32)
        vp0r[:, :64] = vp0.reshape(8, 64)
        vp0r[:, 64] = 1.0
        in_maps.append({
            "qT": np.ascontiguousarray(q[b].T).astype(bf),
            "K3T": k3ts[b],
            "WEK": WEKg,
            "WEV": WEVg,
            "WQT": np.ascontiguousarray(Wq[sl, :].T).astype(bf),
            "WOT": np.ascontiguousarray(Wo[:, sl].T).astype(bf),
            "MASKB": maskb,
            "SEL4": sel4,
            "KP0": kp0.astype(bf),
            "VP0R": vp0r.reshape(1, 520).astype(bf),
        })
    return in_maps


def postprocess(results, bo):
    out = np.zeros((B, T, D), dtype=np.float32)
    for b in range(B):
        out[b] = (np.asarray(results[2 * b]["out_p"], dtype=np.float32)
                  + np.asarray(results[2 * b + 1]["out_p"], dtype=np.float32)
                  + bo[None, :])
    return out


_CACHED_NC = None


def kernel(q, k, v, Wq, Wk, Wv, Wo, bo, conv_w):
    """Full-input entry point. v is unused by the reference computation
    (V is replaced by the conv-compressed K)."""
    global _CACHED_NC
    from concourse.bass_utils import run_bass_kernel_spmd

    q = np.asarray(q, dtype=np.float32)
    k = np.asarray(k, dtype=np.float32)
    Wq = np.asarray(Wq, dtype=np.float32)
    Wk = np.asarray(Wk, dtype=np.float32)
    Wv = np.asarray(Wv, dtype=np.float32)
    Wo = np.asarray(Wo, dtype=np.float32)
    bo = np.asarray(bo, dtype=np.float32)
    conv_w = np.asarray(conv_w, dtype=np.float32)

    in_maps = prep_inputs(q, k, Wq, Wk, Wv, Wo, conv_w)
    if _CACHED_NC is None:
        nc = build_nc()
        nc.finalize()
        _CACHED_NC = nc
    res = run_bass_kernel_spmd(_CACHED_NC, in_maps, list(range(8)))
    return postprocess(res.results, bo)


# revision 68
# speedup vs baseline: 1.0760x; 1.0316x over previous
"""Sparse (conv-compressed) multi-head attention on 8 Trainium2 NeuronCores.

Entry point: kernel(**inputs) -> np.ndarray [4, 2048, 1024] float32.

Sharding: core c = 2*b + g  (b = batch 0..3, g = head-half 0..1).
Each core: batch b, heads [8g, 8g+8), all 2048 queries.
Final projection produces a partial (dv-half contraction); host sums pairs + bias.

v3 design notes:
- The strided conv that builds the compressed keys kc is FUSED into the
  Wk/Wv projections on the host:  kp = (Wk_hh @ Wc3) @ k3,  vp likewise,
  where k3(t) = concat(k[3t], k[3t+1], k[3t+2]).  No conv intermediate,
  no cross-core collective.  Column j=0 (kc[0] = k[0]) comes from the host.
- The host ships kT pre-decimated as K3T[r, i, j] = k[3(j-1)+r, i] so every
  matmul operand is contiguous (strided SBUF reads cost ~1.5x on the PE).
- Score matmuls + exp are decoupled from the attention-value matmuls: scores
  stream into the qp projection passes so the scalar engine's exp pipeline
  (the second-longest serial resource) starts ~60us earlier and never paces
  the PE.  o/normalize/output-projection work is interleaved per q-tile at
  the tail.
- Causal mask applied as a 0/1 bf16 multiply on eT after exp (gpsimd).
"""
import sys
sys.path.insert(0, '/opt/trn_rl_repo')
import numpy as np
import concourse.bass as bass
import concourse.bacc as bacc
import concourse.mybir as mybir
from concourse import tile
from contextlib import ExitStack

f32 = mybir.dt.float32
f32r = mybir.dt.float32r
bf16 = mybir.dt.bfloat16
DT = bf16
Exp = mybir.ActivationFunctionType.Exp

B, T, D, H = 4, 2048, 1024, 16
DH = 64
TC = 683          # compressed keys: 1 + 682
DHALF = D // 2    # per-core head-half width
H8 = H // 2       # heads per core
SCALE = DH ** -0.5  # 0.125

# kp n-splits over keys [1, 683)
KPN = [(1, 341), (342, 341)]

# attention q-tiles (4 x 512) and j-chunks (6 x 128, last = 43 rows)
NJ = 6
JROWS = [128, 128, 128, 128, 128, TC - 5 * 128]  # last = 43
JCS = {m: [jc for jc in range(NJ) if 384 * jc < 512 * (m + 1)] for m in range(4)}
C0 = {(m, jc): max(0, 384 * jc - 512 * m) for m in range(4) for jc in JCS[m]}
# ragged tiles (m, jc) -> first fully-visible column c1 (cols [c0, c1) get mask)
RAGGED = {}
for m in range(4):
    for jc in JCS[m]:
        if not (384 * jc + 381 <= 512 * m):
            RAGGED[(m, jc)] = min(512, 384 * jc + 381 - 512 * m)
RAGGED_LIST = sorted(RAGGED.keys())  # 8 tiles
assert len(RAGGED_LIST) == 8


def chunk_even(lst, n):
    """Split lst into n chunks with sizes as even as possible."""
    k, r = divmod(len(lst), n)
    out, i = [], 0
    for j in range(n):
        sz = k + (1 if j < r else 0)
        out.append(lst[i:i + sz])
        i += sz
    return out


def build_nc():
    nc = bacc.Bacc(None, target_bir_lowering=False, debug=False)

    qT = nc.dram_tensor("qT", [D, T], DT, kind="ExternalInput")
    K3T = nc.dram_tensor("K3T", [3, D, TC], DT, kind="ExternalInput")
    WEK = nc.dram_tensor("WEK", [3 * D, DHALF], DT, kind="ExternalInput")
    WEV = nc.dram_tensor("WEV", [3 * D, DHALF], DT, kind="ExternalInput")
    WQT = nc.dram_tensor("WQT", [D, DHALF], DT, kind="ExternalInput")
    WOT = nc.dram_tensor("WOT", [DHALF, D], DT, kind="ExternalInput")
    MASKB = nc.dram_tensor("MASKB", [8, 128, 512], DT, kind="ExternalInput")
    SEL4 = nc.dram_tensor("SEL4", [4, 128, 128], f32r, kind="ExternalInput")
    KP0 = nc.dram_tensor("KP0", [DHALF, 1], DT, kind="ExternalInput")
    VP0R = nc.dram_tensor("VP0R", [1, 520], DT, kind="ExternalInput")
    OUT = nc.dram_tensor("out_p", [T, D], f32, kind="ExternalOutput")

    with tile.TileContext(nc) as tc, ExitStack() as st:
        st.enter_context(nc.allow_low_precision("bf16 matmuls, f32r recip bcast"))
        pool = lambda **kw: st.enter_context(tc.tile_pool(**kw))
        p_k3 = pool(name="k3", bufs=24)       # K3T chunks [128, 683]
        p_wk = pool(name="wk", bufs=6)        # WEK chunks [128, 512]
        p_wv = pool(name="wv", bufs=6)        # WEV chunks [128, 512]
        p_wq = pool(name="wq", bufs=8)        # WQT chunks [128, 512]
        p_qt = pool(name="qt", bufs=8)        # qT chunks [128, 512]
        p_kpt = pool(name="kpt", bufs=8)      # kp^T per head, K zero-padded
        p_vpa = pool(name="vpa", bufs=6)      # vp_aug [128, 520]
        p_qpt = pool(name="qpt", bufs=4)      # qp^T [128, 2048]
        p_mask = pool(name="mask", bufs=8)    # 0/1 bf16 masks [128, 512]
        p_et = pool(name="et", bufs=42)       # exp(scores) [128, 512] bf16
        p_ont = pool(name="ont", bufs=16)     # unnormalized head outs [128, 512]
        p_wo = pool(name="wo", bufs=8)        # WoT resident [128, 512]
        p_out = pool(name="outsb", bufs=3)    # out staging [128, 1024] f32
        p_rm = pool(name="rm", bufs=4)        # per-m recip denominators [8, 512]
        p_ss = pool(name="ss", bufs=8)        # denom staging rows [1, 512]
        p_small = pool(name="small", bufs=6)

        # kpZ[h]: rows 0-63 = head h's kp dims, rows 64-127 = ZERO.  Scores
        # then contract K=128 (full-rate: K=64 matmuls stream at half speed);
        # the zero rows null out the other head's qp rows in the rhs.
        # head h lives in qpt partition rows [64*(h%2), +64); kpZ mirrors that
        # row placement and zeroes the other head's rows
        kpZ = [p_kpt.tile([128, TC], DT, name="kpt", tag="kpt") for _ in range(H8)]
        for h in range(H8):
            z0 = 64 * ((h + 1) % 2)
            nc.vector.memset(kpZ[h][z0:z0 + 64, :], 0.0)
        vpa = [p_vpa.tile([128, 520], DT, name="vpa", tag="vpa") for _ in range(NJ)]
        qpt = [p_qpt.tile([128, T], DT, name="qpt", tag="qpt") for _ in range(4)]
        o_nt = {(kk, m): p_ont.tile([128, 512], DT, name="ont", tag="ont")
                for kk in range(4) for m in range(4)}
        ones_vpa = p_small.tile([128, 8], f32, name="ones_vpa", tag="ones_vpa",
                                bufs=1)
        nc.vector.memset(ones_vpa[:], 1.0)

        # ---- kp^T fused: kpt[m][:, j] = (Wk_hh @ Wc3) @ k3(j-1), j in [1, 683)
        k3 = {}
        k3q = [nc.sync, nc.scalar, nc.gpsimd]  # parallel issue for fast start
        with tc.tile_pool(name="ps_kp", bufs=8, space="PSUM") as ps_pool:
            ps_kp = {}
            for ic in range(8):
                for r in range(3):
                    kt = p_k3.tile([128, TC], DT, name="k3", tag="k3")
                    wt = p_wk.tile([128, DHALF], DT, name="wk", tag="wk")
                    # wek first on gpsimd: k3(0,2) isn't needed until the
                    # third chunk block, wek(0,0) gates the first matmul
                    nc.gpsimd.dma_start(
                        wt[:], WEK[1024 * r + 128 * ic:1024 * r + 128 * (ic + 1), :])
                    eng = k3q[r] if ic == 0 else nc.sync
                    eng.dma_start(kt[:], K3T[r, 128 * ic:128 * (ic + 1), :])
                    k3[(ic, r)] = kt
                    first = (ic == 0 and r == 0)
                    last = (ic == 7 and r == 2)
                    for m in range(4):
                        for ni, (t0, tw) in enumerate(KPN):
                            if first:
                                ps_kp[(m, ni)] = ps_pool.tile(
                                    [128, 341], f32, name="ps_kp", tag="ps_kp")
                            nc.tensor.matmul(
                                ps_kp[(m, ni)][:, :tw],
                                wt[:, 128 * m:128 * (m + 1)],
                                kt[:, t0:t0 + tw],
                                start=first, stop=last)
            for (m, ni), ps in ps_kp.items():
                t0, tw = KPN[ni]
                # feature rows [0:64] = head 2m, [64:128] = head 2m+1; rows
                # keep their partition placement (no shift needed)
                nc.scalar.copy(kpZ[2 * m][0:64, t0:t0 + tw], ps[0:64, :tw])
                nc.vector.tensor_copy(
                    kpZ[2 * m + 1][64:128, t0:t0 + tw], ps[64:128, :tw])

        # ---- small loads on the scalar queue, issued after the kp phase so
        # they don't compete with K3T/WEK for HBM bandwidth at startup
        mk = []
        for ti in range(8):
            mt = p_mask.tile([128, 512], DT, name="mask", tag="mask")
            nc.scalar.dma_start(mt[:], MASKB[ti])
            mk.append(mt)
        sel = []
        for t4 in range(4):
            s_t = p_small.tile([128, 128], f32r, name="sel", tag="sel", bufs=4)
            nc.scalar.dma_start(s_t[:], SEL4[t4])
            sel.append(s_t)
        wot = {}
        for kk in range(4):
            for nn2 in range(2):
                wt = p_wo.tile([128, 512], DT, name="wo", tag="wo")
                nc.scalar.dma_start(
                    wt[:], WOT[128 * kk:128 * (kk + 1), 512 * nn2:512 * (nn2 + 1)])
                wot[(kk, nn2)] = wt
        vp0 = p_small.tile([1, 520], DT, name="vp0", tag="vp0", bufs=1)
        nc.scalar.dma_start(vp0[:], VP0R[:])
        with nc.allow_non_contiguous_dma(reason="kp col-0 writes, 1KB total"):
            for h in range(H8):
                off = 64 * (h % 2)
                nc.scalar.dma_start(
                    kpZ[h][off:off + 64, 0:1], KP0[64 * h:64 * (h + 1), :])

        # ---- vp_aug fused: vpa[jb][j, 65h+c] = vp[128jb+j, 64h+c], col 65h+64=1
        with tc.tile_pool(name="ps_vp", bufs=6, space="PSUM") as ps_pool:
            ps_vp = {}
            for ic in range(8):
                for r in range(3):
                    wt = p_wv.tile([128, DHALF], DT, name="wv", tag="wv")
                    nc.gpsimd.dma_start(
                        wt[:], WEV[1024 * r + 128 * ic:1024 * r + 128 * (ic + 1), :])
                    first = (ic == 0 and r == 0)
                    last = (ic == 7 and r == 2)
                    for jb in range(NJ):
                        jr = JROWS[jb]
                        if first:
                            ps_vp[jb] = ps_pool.tile(
                                [128, 512], f32, name="ps_vp", tag="ps_vp")
                        nc.tensor.matmul(
                            ps_vp[jb][:jr, :],
                            k3[(ic, r)][:, 128 * jb:128 * jb + jr],
                            wt[:],
                            start=first, stop=last)
            for jb in range(NJ):
                jr = JROWS[jb]
                dst = vpa[jb][:jr].rearrange("p (h c) -> p h c", c=65)
                src = ps_vp[jb][:jr].rearrange("p (h c) -> p h c", c=64)
                if jb % 2 == 0:
                    nc.scalar.copy(dst[:, :, 0:64], src[:])
                else:
                    nc.vector.tensor_copy(dst[:, :, 0:64], src[:])
                nc.vector.tensor_copy(
                    dst[:, :, 64:65],
                    ones_vpa[:jr, :].rearrange("p (h c) -> p h c", c=1))
            # row 0 (key 0) comes from the host: kc[0] = k[0]
            nc.vector.tensor_copy(vpa[0][0:1, :], vp0[:])

        # ---- attention helpers (scores decoupled from o) ----
        ET = {}

        def emit_score(m, h, jc, ps_sc, s_bufs=2):
            hc = h // 2
            jr, c0 = JROWS[jc], C0[(m, jc)]
            ps = ps_sc.tile([128, 512], f32, name="ps_s", tag="ps_s",
                            bufs=s_bufs)
            nc.tensor.matmul(
                ps[:jr, c0:],
                kpZ[h][:, 128 * jc:128 * jc + jr],
                qpt[hc][:, 512 * m + c0:512 * (m + 1)],
                start=True, stop=True)
            et = p_et.tile([128, 512], DT, name="et", tag="et")
            nc.scalar.activation(et[:jr, c0:], ps[:jr, c0:], Exp, scale=SCALE)
            if (m, jc) in RAGGED:
                c1 = RAGGED[(m, jc)]
                ti = RAGGED_LIST.index((m, jc))
                nc.gpsimd.tensor_mul(
                    et[:jr, c0:c1], et[:jr, c0:c1], mk[ti][:jr, c0:c1])
            ET[(m, h, jc)] = et

        # rm zero-padded to K=128 for the full-rate bc matmul (sel rows >= 8
        # are zero, but NaN garbage in rm rows would still poison 0*NaN)
        rm = [p_rm.tile([128, 512], f32r, name="rm", tag="rm") for _ in range(4)]
        for m in range(4):
            nc.vector.memset(rm[m][:].bitcast(f32), 0.0)
        S_m = [p_rm.tile([8, 512], f32, name="sm", tag="sm") for _ in range(4)]

        def emit_o_mm(m, h, jc, po, first, last):
            jr, c0 = JROWS[jc], C0[(m, jc)]
            nc.tensor.matmul(
                po[:65, c0:],
                vpa[jc][:jr, 65 * h:65 * (h + 1)],
                ET[(m, h, jc)][:jr, c0:],
                start=first, stop=last)

        def emit_o_post(m, h, po):
            hc, off = h // 2, (h % 2) * 64
            ss = p_ss.tile([1, 512], f32, name="ss", tag="ss", bufs=2)
            nc.vector.tensor_copy(ss[:], po[64:65, :])
            nc.sync.dma_start(S_m[m][h:h + 1, :], ss[:])
            if h == H8 - 1:
                # recip gates bc(m); run it ahead of the (non-urgent) o_nt copy
                rt = p_ss.tile([8, 512], f32, name="rt", tag="rt", bufs=1)
                nc.vector.reciprocal_approx_fast(out=rt[:], in_=S_m[m][:])
                nc.vector.tensor_copy(rm[m][0:8, :], rt[:])
            nc.vector.tensor_copy(o_nt[(hc, m)][off:off + 64, :], po[0:64, :])

        def emit_o(m, h, ps_att, o_bufs=2):
            js = JCS[m]
            po = ps_att.tile([128, 512], f32, name="ps_o", tag="ps_o",
                             bufs=o_bufs)
            for jc in js:
                emit_o_mm(m, h, jc, po, jc == js[0], jc == js[-1])
            emit_o_post(m, h, po)

        def emit_bc(m, ps_att):
            for t4 in range(4):
                bc = ps_att.tile([128, 512], f32, name="ps_bc", tag="ps_bc",
                                 bufs=2)
                nc.tensor.matmul(bc[:], sel[t4][:], rm[m][:], start=True,
                                 stop=True)
                for half in range(2):
                    dst = o_nt[(t4, m)][64 * half:64 * half + 64, :]
                    nc.vector.tensor_mul(
                        dst, dst, bc[64 * half:64 * half + 64, :])

        def emit_fin_mq(m, mq, ps_att):
            ob = p_out.tile([128, D], f32, name="outsb", tag="outsb")
            rows = slice(512 * m + 128 * mq, 512 * m + 128 * (mq + 1))
            for nn2 in range(2):
                pf = ps_att.tile([128, 512], f32, name="ps_f", tag="ps_f",
                                 bufs=2)
                for kk in range(4):
                    nc.tensor.matmul(
                        pf[:],
                        o_nt[(kk, m)][:, 128 * mq:128 * (mq + 1)],
                        wot[(kk, nn2)][:],
                        start=(kk == 0), stop=(kk == 3))
                # scalar still runs the exp stream while m=0/1 finalize
                if m < 2:
                    nc.vector.tensor_copy(ob[:, 512 * nn2:512 * (nn2 + 1)], pf[:])
                else:
                    nc.scalar.copy(ob[:, 512 * nn2:512 * (nn2 + 1)], pf[:])
            dq = (nc.gpsimd, nc.sync)[mq % 2]
            dq.dma_start(OUT[rows, :], ob[:])

        # ---- qp passes; scores(m=pass-1) and o(pass-2) interleaved ----
        # PSUM per pass: ps_qp 4 + ps_s 2 + ps_o 2 = 8 banks.
        with tc.tile_pool(name="ps_qp", bufs=8, space="PSUM") as ps_pool:
            for npass in range(4):
                sc_chunks = [[] for _ in range(8)]
                if npass >= 1:
                    msc = npass - 1
                    tiles = [(h, jc) for h in range(H8) for jc in JCS[msc]]
                    sc_chunks = chunk_even(tiles, 8)
                ps_qp = {}
                for kk in range(8):
                    wq_t = p_wq.tile([128, DHALF], DT, name="wq", tag="wq")
                    nc.sync.dma_start(wq_t[:], WQT[128 * kk:128 * (kk + 1), :])
                    qt_t = p_qt.tile([128, 512], DT, name="qt", tag="qt")
                    nc.sync.dma_start(
                        qt_t[:],
                        qT[128 * kk:128 * (kk + 1), 512 * npass:512 * (npass + 1)])
                    # zip qp matmuls with score tiles so the exp stream always
                    # has non-dependent PE work between score matmuls
                    sc_it = list(sc_chunks[kk])
                    for m in range(4):
                        if kk == 0:
                            ps_qp[m] = ps_pool.tile(
                                [128, 512], f32, name="ps_qp", tag="ps_qp",
                                bufs=4)
                        nc.tensor.matmul(
                            ps_qp[m][:],
                            wq_t[:, 128 * m:128 * (m + 1)],
                            qt_t[:],
                            start=(kk == 0), stop=(kk == 7))
                        if sc_it:
                            h, jc = sc_it.pop(0)
                            emit_score(npass - 1, h, jc, ps_pool)
                    for (h, jc) in sc_it:
                        emit_score(npass - 1, h, jc, ps_pool)
                    if npass >= 2:
                        emit_o(npass - 2, kk, ps_pool)
                for m, ps in ps_qp.items():
                    # scalar runs the exp stream during passes >= 1; keep the
                    # casts off its queue there
                    if npass == 0 and m % 2 == 0:
                        nc.scalar.copy(
                            qpt[m][:, 512 * npass:512 * (npass + 1)], ps[:])
                    else:
                        nc.vector.tensor_copy(
                            qpt[m][:, 512 * npass:512 * (npass + 1)], ps[:])

        # ---- tail: sc(3) / o(2) / o(3) / finalize pipelined per head ----
        # PSUM: ps_s 2 + ps_o 2 + ps_bc 2 + ps_f 2 = 8 banks.
        with tc.tile_pool(name="ps_att", bufs=2, space="PSUM") as ps_att:
            fin_chunks = []  # deferred finalize chunks for m = 0, 1
            fin_chunks.append(lambda: emit_bc(0, ps_att))
            for mq in range(4):
                fin_chunks.append(lambda m=0, q=mq: emit_fin_mq(m, q, ps_att))
            fin_chunks.append(lambda: emit_bc(1, ps_att))
            for mq in range(4):
                fin_chunks.append(lambda m=1, q=mq: emit_fin_mq(m, q, ps_att))
            ci = 0
            for h in range(H8):
                # zip sc3 / o2 / o3 matmuls so exp latency hides behind
                # independent o-matmuls instead of gating ps_s recycling
                js3, js2 = JCS[3], JCS[2]
                po2 = ps_att.tile([128, 512], f32, name="ps_o", tag="ps_o",
                                  bufs=2)
                emit_score(3, h, js3[0], ps_att)
                emit_score(3, h, js3[1], ps_att)
                for i, jc in enumerate(js2):
                    emit_o_mm(2, h, jc, po2, i == 0, i == len(js2) - 1)
                emit_score(3, h, js3[2], ps_att)
                emit_score(3, h, js3[3], ps_att)
                if h >= 1:
                    po3 = ps_att.tile([128, 512], f32, name="ps_o",
                                      tag="ps_o", bufs=2)
                    for i, jc in enumerate(js3):
                        emit_o_mm(3, h - 1, jc, po3, i == 0, i == 5)
                emit_score(3, h, js3[4], ps_att)
                emit_score(3, h, js3[5], ps_att)
                emit_o_post(2, h, po2)
                if h >= 1:
                    emit_o_post(3, h - 1, po3)
                # 1-2 finalize chunks per slot; bc(1) needs o(1) (done in qp
                # pass 3), all fit behind the sc3/o2/o3 stream
                for _ in range(2 if h >= 4 else 1):
                    if ci < len(fin_chunks):
                        fin_chunks[ci]()
                        ci += 1
            while ci < len(fin_chunks):
                fin_chunks[ci]()
                ci += 1
            emit_o(3, 7, ps_att)
            emit_bc(2, ps_att)
            emit_fin_mq(2, 0, ps_att)
            emit_fin_mq(2, 1, ps_att)
            # fin(3): all 4 bc tiles live at once (borrow the now-idle ps_s /
            # ps_o rings), normalize per 128-col slice, and interleave with
            # the remaining fin(2) chunks so mq3 starts as early as possible
            bc3 = []
            for t4, tag in enumerate(("ps_bc", "ps_bc", "ps_s", "ps_o")):
                bc = ps_att.tile([128, 512], f32, name="ps_bc3", tag=tag,
                                 bufs=2)
                nc.tensor.matmul(bc[:], sel[t4][:], rm[3][:], start=True,
                                 stop=True)
                bc3.append(bc)
            emit_fin_mq(2, 2, ps_att)

            def norm3(mq):
                for t4 in range(4):
                    for half in range(2):
                        dst = o_nt[(t4, 3)][64 * half:64 * half + 64,
                                            128 * mq:128 * (mq + 1)]
                        nc.vector.tensor_mul(
                            dst, dst,
                            bc3[t4][64 * half:64 * half + 64,
                                    128 * mq:128 * (mq + 1)])

            norm3(0)
            emit_fin_mq(2, 3, ps_att)
            norm3(1)
            emit_fin_mq(3, 0, ps_att)
            norm3(2)
            emit_fin_mq(3, 1, ps_att)
            norm3(3)
            emit_fin_mq(3, 2, ps_att)
            emit_fin_mq(3, 3, ps_att)

    return nc


def make_maskb():
    import ml_dtypes
    mask = np.zeros((8, 128, 512), dtype=np.float32)
    for t, (m, jc) in enumerate(RAGGED_LIST):
        qq = 512 * m + np.arange(512)[None, :]
        jj = 128 * jc + np.arange(128)[:, None]
        mask[t] = (3 * jj <= qq).astype(np.float32)  # 1.0 where visible
    return mask.astype(ml_dtypes.bfloat16)


def make_sel4():
    sel = np.zeros((4, 128, 128), dtype=np.float32)
    for t in range(4):
        for r in range(128):
            sel[t, 2 * t + r // 64, r] = 1.0
    return sel


def make_k3t(kb):
    """kb: [T, D] f32 for one batch -> K3T [3, D, TC] bf16.
    K3T[r, i, j] = k[3(j-1)+r, i] for j >= 1; column 0 is zero."""
    import ml_dtypes
    out = np.zeros((3, D, TC), dtype=np.float32)
    for r in range(3):
        out[r, :, 1:] = kb[r:r + 3 * (TC - 1):3, :].T
    return out.astype(ml_dtypes.bfloat16)


def prep_inputs(q, k, Wq, Wk, Wv, Wo, conv_w):
    """Returns list of 8 in_maps (core c = 2b + g)."""
    import ml_dtypes
    bf = ml_dtypes.bfloat16
    # Wc3T[r*1024 + i, o] = conv_w[o, i, r]  (so kc[j] = Wc3T.T @ k3(j-1))
    Wc3T = np.ascontiguousarray(
        conv_w.transpose(2, 1, 0).reshape(3 * D, D)).astype(np.float32)
    maskb = make_maskb()
    sel4 = make_sel4()
    halves = []
    for g in range(2):
        sl = slice(DHALF * g, DHALF * (g + 1))
        WEKg = np.ascontiguousarray((Wc3T @ Wk[sl].T)).astype(bf)
        WEVg = np.ascontiguousarray((Wc3T @ Wv[sl].T)).astype(bf)
        halves.append((sl, WEKg, WEVg))
    k3ts = [make_k3t(k[b]) for b in range(B)]
    in_maps = []
    for c in range(8):
        b, g = c // 2, c % 2
        sl, WEKg, WEVg = halves[g]
        kp0 = (Wk[sl] @ k[b, 0]).astype(np.float32).reshape(DHALF, 1)
        vp0 = (Wv[sl] @ k[b, 0]).astype(np.float32)
        vp0r = np.zeros((8, 65), np.float32)
        vp0r[:, :64] = vp0.reshape(8, 64)
        vp0r[:, 64] = 1.0
        in_maps.append({
            "qT": np.ascontiguousarray(q[b].T).astype(bf),
            "K3T": k3ts[b],
            "WEK": WEKg,
            "WEV": WEVg,
            "WQT": np.ascontiguousarray(Wq[sl, :].T).astype(bf),
            "WOT": np.ascontiguousarray(Wo[:, sl].T).astype(bf),
            "MASKB": maskb,
            "SEL4": sel4,
            "KP0": kp0.astype(bf),
            "VP0R": vp0r.reshape(1, 520).astype(bf),
        })
    return in_maps


def postprocess(results, bo):
    out = np.zeros((B, T, D), dtype=np.float32)
    for b in range(B):
        out[b] = (np.asarray(results[2 * b]["out_p"], dtype=np.float32)
                  + np.asarray(results[2 * b + 1]["out_p"], dtype=np.float32)
                  + bo[None, :])
    return out


_CACHED_NC = None


def kernel(q, k, v, Wq, Wk, Wv, Wo, bo, conv_w):
    """Full-input entry point. v is unused by the reference computation
    (V is replaced by the conv-compressed K)."""
    global _CACHED_NC
    from concourse.bass_utils import run_bass_kernel_spmd

    q = np.asarray(q, dtype=np.float32)
    k = np.asarray(k, dtype=np.float32)
    Wq = np.asarray(Wq, dtype=np.float32)
    Wk = np.asarray(Wk, dtype=np.float32)
    Wv = np.asarray(Wv, dtype=np.float32)
    Wo = np.asarray(Wo, dtype=np.float32)
    bo = np.asarray(bo, dtype=np.float32)
    conv_w = np.asarray(conv_w, dtype=np.float32)

    in_maps = prep_inputs(q, k, Wq, Wk, Wv, Wo, conv_w)
    if _CACHED_NC is None:
        nc = build_nc()
        nc.finalize()
        _CACHED_NC = nc
    res = run_bass_kernel_spmd(_CACHED_NC, in_maps, list(range(8)))
    return postprocess(res.results, bo)
